# revision 11
# baseline (speedup 1.0000x reference)
"""BiAffine layer kernel for 8 Trainium2 NeuronCores.

Reference computation (per batch b):
  s = relu(x @ sW.T + sb)                  [L, E]
  t = relu(x @ tW.T + tb)                  [L, E]
  key = (s @ blW.T).reshape(L, E, N)
  out1[i, n, l] = sum_e key[i, e, n] * t[l, e]
  su = s @ Wu.T ; tv = t @ Wv.T            (Wu, Wv = f2W[:, :E], f2W[:, E:])
  h[i, j, :] = relu(su[i] + tv[j] + f2b)
  out2[i, n, j] = sum_e h[i, j, e] * f3W[n, e] + f3b[n]
  out = out1 + out2                        [L, N, L]

Sharding: 8 cores = 2 batches x 4 blocks of 128 source positions (i).
Each core computes the full t/tv for its batch (cheap) and its own
128-row slab of the output [128, N, L].

On-chip layout: everything transposed so the contraction dim (E or H) is
on partitions.  Main loop processes i in quads; for each quad one PSUM
bank [128, 512] holds 4 outputs at 32-aligned partition offsets
(rows 32k+n), written by M=12 matmuls in 4 distinct PE column groups
(tile concurrency).  h is produced by ACT/DVE/GPSIMD tensor ops
(relu with per-partition scalar bias).
"""

import sys

sys.path.insert(0, "/opt/trn_rl_repo")

import numpy as np

B, L, H, E, N = 2, 512, 768, 256, 12
EC = E // 128  # 2 e-chunks
HC = H // 128  # 6 h-chunks
IB = L // 4  # 128 i's per core
NCORES = 8
QUADS = IB // 4  # 32

_cache = {}


def build_nc():
    import concourse.bass as bass
    import concourse.tile as tile
    from concourse import bacc, mybir
    from contextlib import ExitStack

    fp32 = mybir.dt.float32
    AF = mybir.ActivationFunctionType
    ALU = mybir.AluOpType

    nc = bacc.Bacc("TRN2")

    # ---- I/O ----
    xT = nc.dram_tensor("xT", [H, L], fp32, kind="ExternalInput")
    xTi = nc.dram_tensor("xTi", [H, IB], fp32, kind="ExternalInput")
    sWT = nc.dram_tensor("sWT", [H, E], fp32, kind="ExternalInput")
    tWT = nc.dram_tensor("tWT", [H, E], fp32, kind="ExternalInput")
    WuT = nc.dram_tensor("WuT", [E, E], fp32, kind="ExternalInput")
    WvT = nc.dram_tensor("WvT", [E, E], fp32, kind="ExternalInput")
    blWT = nc.dram_tensor("blWT", [E, E * N], fp32, kind="ExternalInput")
    f3WT = nc.dram_tensor("f3WT", [E, N], fp32, kind="ExternalInput")
    sb_r = nc.dram_tensor("sb_r", [128, EC], fp32, kind="ExternalInput")
    tb_r = nc.dram_tensor("tb_r", [128, EC], fp32, kind="ExternalInput")
    f2b_r = nc.dram_tensor("f2b_r", [128, EC], fp32, kind="ExternalInput")
    f3b128 = nc.dram_tensor("f3b128", [128, 1], fp32, kind="ExternalInput")
    out = nc.dram_tensor("out", [IB, N, L], fp32, kind="ExternalOutput")

    with tile.TileContext(nc) as tc, ExitStack() as ctx:
        consts = ctx.enter_context(tc.tile_pool(name="consts", bufs=1))
        acts = ctx.enter_context(tc.tile_pool(name="acts", bufs=1))

        # ---- DMA in ----
        def load(pool, src, shape, name):
            t = pool.tile(shape, fp32, name=name)
            nc.sync.dma_start(out=t[:], in_=src)
            return t

        xT_sb = [load(consts, xT[128 * c : 128 * (c + 1), :], [128, L], f"xT{c}")
                 for c in range(HC)]
        xTi_sb = [load(consts, xTi[128 * c : 128 * (c + 1), :], [128, IB], f"xTi{c}")
                  for c in range(HC)]
        sWT_sb = [load(consts, sWT[128 * c : 128 * (c + 1), :], [128, E], f"sWT{c}")
                  for c in range(HC)]
        tWT_sb = [load(consts, tWT[128 * c : 128 * (c + 1), :], [128, E], f"tWT{c}")
                  for c in range(HC)]
        WuT_sb = [load(consts, WuT[128 * c : 128 * (c + 1), :], [128, E], f"WuT{c}")
                  for c in range(EC)]
        WvT_sb = [load(consts, WvT[128 * c : 128 * (c + 1), :], [128, E], f"WvT{c}")
                  for c in range(EC)]
        blWT_sb = [load(consts, blWT[128 * c : 128 * (c + 1), :], [128, E * N], f"blWT{c}")
                   for c in range(EC)]
        f3WT_sb = [load(consts, f3WT[128 * c : 128 * (c + 1), :], [128, N], f"f3WT{c}")
                   for c in range(EC)]
        sb_sb = load(consts, sb_r[:], [128, EC], "sb_sb")
        tb_sb = load(consts, tb_r[:], [128, EC], "tb_sb")
        f2b_sb = load(consts, f2b_r[:], [128, EC], "f2b_sb")
        f3b_sb = load(consts, f3b128[:], [128, 1], "f3b_sb")

        # ---- prep: tT, sT, tvT, suT, keyE32 ----
        tT_sb, sT_sb, tvT_sb, suT_sb, keyE32_sb = [], [], [], [], []
        for ec in range(EC):
            tT_sb.append(acts.tile([128, L], fp32, name=f"tT{ec}"))
            sT_sb.append(acts.tile([128, IB], fp32, name=f"sT{ec}"))
            tvT_sb.append(acts.tile([128, L], fp32, name=f"tvT{ec}"))
            suT_sb.append(acts.tile([128, IB], fp32, name=f"suT{ec}"))
            keyE32_sb.append(acts.tile([128, 32 * IB], fp32, name=f"keyE32_{ec}"))
            # zero the pad columns (cols 32i+12..32i+31) so the M=32
            # start-matmuls read finite weights
            nc.gpsimd.memset(keyE32_sb[ec][:], 0.0)

        with tc.tile_pool(name="prep_psum", bufs=3, space="PSUM") as pp:
            for ec in range(EC):
                # tT[e,j] = relu(sum_hc tWT[hc][:, ec*128:+128].T @ xT[hc] + tb)
                ps_t = pp.tile([128, L], fp32, name="ps_t", tag="ps")
                for hc in range(HC):
                    nc.tensor.matmul(
                        ps_t[:],
                        lhsT=tWT_sb[hc][:, 128 * ec : 128 * (ec + 1)],
                        rhs=xT_sb[hc][:],
                        start=(hc == 0),
                        stop=(hc == HC - 1),
                    )
                nc.scalar.activation(tT_sb[ec][:], ps_t[:], AF.Relu,
                                     bias=tb_sb[:, ec : ec + 1])

                ps_s = pp.tile([128, L], fp32, name="ps_s", tag="ps")
                for hc in range(HC):
                    nc.tensor.matmul(
                        ps_s[:, :IB],
                        lhsT=sWT_sb[hc][:, 128 * ec : 128 * (ec + 1)],
                        rhs=xTi_sb[hc][:],
                        start=(hc == 0),
                        stop=(hc == HC - 1),
                    )
                nc.scalar.activation(sT_sb[ec][:], ps_s[:, :IB], AF.Relu,
                                     bias=sb_sb[:, ec : ec + 1])

            for ec in range(EC):
                # tvT[e,j] = sum_epc WvT[epc][:, ec-chunk].T @ tT[epc]
                ps_tv = pp.tile([128, L], fp32, name="ps_tv", tag="ps")
                for epc in range(EC):
                    nc.tensor.matmul(
                        ps_tv[:],
                        lhsT=WvT_sb[epc][:, 128 * ec : 128 * (ec + 1)],
                        rhs=tT_sb[epc][:],
                        start=(epc == 0),
                        stop=(epc == EC - 1),
                    )
                nc.scalar.copy(tvT_sb[ec][:], ps_tv[:])

                # suT[e,i] = sum_epc WuT.T @ sT + f2b
                ps_su = pp.tile([128, L], fp32, name="ps_su", tag="ps")
                for epc in range(EC):
                    nc.tensor.matmul(
                        ps_su[:, :IB],
                        lhsT=WuT_sb[epc][:, 128 * ec : 128 * (ec + 1)],
                        rhs=sT_sb[epc][:],
                        start=(epc == 0),
                        stop=(epc == EC - 1),
                    )
                nc.scalar.activation(suT_sb[ec][:], ps_su[:, :IB], AF.Identity,
                                     bias=f2b_sb[:, ec : ec + 1])

            # key: keyE32[ec][e_local, 32*i + n] = key[i, 128*ec + e_local, n]
            #    = sum_e' blW[(128*ec+e)*12 + n, e'] * s[i, e']
            # kxm = blWT[e'c][:, (ec*128*12 + n) + 12*e for e in 0..127]
            blWT3 = [blWT_sb[c].rearrange("p (e n) -> p e n", n=N) for c in range(EC)]
            key32 = [keyE32_sb[c].rearrange("p (i w) -> p i w", w=32) for c in range(EC)]
            for ec in range(EC):
                for n in range(N):
                    ps_k = pp.tile([128, L], fp32, name="ps_k", tag="ps")
                    for epc in range(EC):
                        nc.tensor.matmul(
                            ps_k[:, :IB],
                            lhsT=blWT3[epc][:, 128 * ec : 128 * (ec + 1), n],
                            rhs=sT_sb[epc][:],
                            start=(epc == 0),
                            stop=(epc == EC - 1),
                        )
                    nc.vector.tensor_copy(out=key32[ec][:, :, n], in_=ps_k[:, :IB])

        # ---- main loop over quads ----
        # engine schedule for the 8 h-ops per quad (k, ec):
        # 3x vector, 3x scalar, 2x gpsimd
        ENG = ["v", "s", "g", "v", "s", "g", "v", "s"]

        hp = ctx.enter_context(tc.tile_pool(name="hp", bufs=8))
        outp = ctx.enter_context(tc.tile_pool(name="outp", bufs=3))
        mp = ctx.enter_context(tc.tile_pool(name="main_psum", bufs=4, space="PSUM"))

        for g in range(QUADS):
            ps = mp.tile([128, L], fp32, name="ps")
            # out1 first: only needs keyE32 + tT.  The ec==0 matmul is
            # M=32 with start=True: it initializes the full 32-row group
            # (12 real rows + 20 pad rows from the zeroed keyE32 pads) so
            # every partition of ps is written before the final read.
            for ec in range(EC):
                for k in range(4):
                    i = 4 * g + k
                    M_ = 32 if ec == 0 else N
                    nc.tensor.matmul(
                        ps[32 * k : 32 * k + M_, :],
                        lhsT=keyE32_sb[ec][:, 32 * i : 32 * i + M_],
                        rhs=tT_sb[ec][:],
                        start=(ec == 0),
                        stop=False,
                        tile_position=(0, 32 * k),
                        skip_group_check=True,
                    )
            # h + out2
            hs = []
            for k in range(4):
                i = 4 * g + k
                for ec in range(EC):
                    h = hp.tile([128, L], fp32, name=f"h{len(hs) % 8}", tag="h")
                    eng = ENG[len(hs) % 8]
                    su_ap = suT_sb[ec][:, i : i + 1]
                    if eng == "s":
                        nc.scalar.activation(h[:], tvT_sb[ec][:], AF.Relu, bias=su_ap)
                    else:
                        e = nc.vector if eng == "v" else nc.gpsimd
                        e.tensor_scalar(h[:], tvT_sb[ec][:], su_ap, 0.0,
                                        ALU.add, ALU.max)
                    hs.append((k, ec, h))
            for k, ec, h in hs:
                nc.tensor.matmul(
                    ps[32 * k : 32 * k + N, :],
                    lhsT=f3WT_sb[ec][:],
                    rhs=h[:],
                    start=False,
                    stop=(ec == EC - 1),
                    tile_position=(0, 32 * k),
                    skip_group_check=True,
                )
            # finalize: add f3b, copy PSUM->SBUF, DMA out
            ob = outp.tile([128, L], fp32, name="ob")
            nc.vector.tensor_scalar(ob[:], ps[:], f3b_sb[:, 0:1], None, ALU.add)
            for k in range(4):
                nc.sync.dma_start(out=out[4 * g + k, :, :],
                                  in_=ob[32 * k : 32 * k + N, :])

    nc.compile()
    return nc


def _get_nc():
    if "nc" not in _cache:
        _cache["nc"] = build_nc()
    return _cache["nc"]


def _make_in_maps(inputs):
    x = np.asarray(inputs["x"], np.float32)
    f32c = lambda a: np.ascontiguousarray(np.asarray(a, np.float32))

    f2W = np.asarray(inputs["f2W"], np.float32)
    shared = {
        "sWT": f32c(inputs["sW"].T),
        "tWT": f32c(inputs["tW"].T),
        "WuT": f32c(f2W[:, :E].T),
        "WvT": f32c(f2W[:, E:].T),
        "blWT": f32c(inputs["blW"].T),
        "f3WT": f32c(inputs["f3W"].T),
        "sb_r": f32c(np.asarray(inputs["sb"], np.float32).reshape(EC, 128).T),
        "tb_r": f32c(np.asarray(inputs["tb"], np.float32).reshape(EC, 128).T),
        "f2b_r": f32c(np.asarray(inputs["f2b"], np.float32).reshape(EC, 128).T),
    }
    f3b128 = np.zeros((128, 1), np.float32)
    for k in range(4):
        f3b128[32 * k : 32 * k + N, 0] = np.asarray(inputs["f3b"], np.float32)
    shared["f3b128"] = f3b128

    in_maps = []
    xT_by_b = [f32c(x[b].T) for b in range(B)]
    for c in range(NCORES):
        b, r = divmod(c, 4)
        m = dict(shared)
        m["xT"] = xT_by_b[b]
        m["xTi"] = f32c(x[b, IB * r : IB * (r + 1), :].T)
        in_maps.append(m)
    return in_maps


def _gather(results):
    full = np.empty((B, L, N, L), np.float32)
    for c in range(NCORES):
        b, r = divmod(c, 4)
        full[b, IB * r : IB * (r + 1)] = results[c]["out"]
    return full


def kernel(x, sW, sb, tW, tb, f2W, f2b, f3W, f3b, blW):
    from concourse.bass_utils import run_bass_kernel_spmd

    in_maps = _make_in_maps(dict(
        x=x, sW=sW, sb=sb, tW=tW, tb=tb, f2W=f2W, f2b=f2b,
        f3W=f3W, f3b=f3b, blW=blW,
    ))
    nc = _get_nc()
    res = run_bass_kernel_spmd(nc, in_maps, core_ids=list(range(NCORES)))
    return _gather(res.results)


# revision 13
# speedup vs baseline: 3.1132x; 3.1132x over previous
"""BiAffine layer kernel for 8 Trainium2 NeuronCores.

Reference computation (per batch b):
  s = relu(x @ sW.T + sb)                  [L, E]
  t = relu(x @ tW.T + tb)                  [L, E]
  key = (s @ blW.T).reshape(L, E, N)
  out1[i, n, l] = sum_e key[i, e, n] * t[l, e]
  su = s @ Wu.T ; tv = t @ Wv.T            (Wu, Wv = f2W[:, :E], f2W[:, E:])
  h[i, j, :] = relu(su[i] + tv[j] + f2b)
  out2[i, n, j] = sum_e h[i, j, e] * f3W[n, e] + f3b[n]
  out = out1 + out2                        [L, N, L]

Sharding: 8 cores = 2 batches x 4 blocks of 128 source positions (i).

Main loop: i in quads; one PSUM bank [128, 512] holds 4 outputs at
32-aligned partition offsets (rows 32k+n), M=12/32 matmuls in 4 distinct
PE column groups (tile concurrency).

h production per lane k = i%4:
  k=0,1 -> DVE, k=2 -> GPSIMD: h' = max(tv, -su') via one bf16
    tensor_tensor with a free-broadcast AP ([128,1024] = both e-chunks).
    h' = relu(tv+su') - su', so the matmul result is short by
    sum_e f3W[n,e]*su'[e,i]; that rank-1 term C is precomputed once
    (PE) and folded into the final copy's per-partition bias.
  k=3 -> ACT: true h = relu(tv + su') via activation bias (native fast).
Final copy: ACT activation(Identity, bias = C4[:, g] + f3b pattern).
"""

import sys

sys.path.insert(0, "/opt/trn_rl_repo")

import numpy as np

B, L, H, E, N = 2, 512, 768, 256, 12
EC = E // 128  # 2 e-chunks
HC = H // 128  # 6 h-chunks
IB = L // 4  # 128 i's per core
NCORES = 8
QUADS = IB // 4  # 32

_cache = {}


def build_nc():
    import concourse.bass as bass
    import concourse.tile as tile
    from concourse import bacc, mybir
    from contextlib import ExitStack

    fp32 = mybir.dt.float32
    bf16 = mybir.dt.bfloat16
    AF = mybir.ActivationFunctionType
    ALU = mybir.AluOpType

    nc = bacc.Bacc("TRN2")

    # ---- I/O ----
    xT = nc.dram_tensor("xT", [H, L], fp32, kind="ExternalInput")
    xTi = nc.dram_tensor("xTi", [H, IB], fp32, kind="ExternalInput")
    sWT = nc.dram_tensor("sWT", [H, E], fp32, kind="ExternalInput")
    tWT = nc.dram_tensor("tWT", [H, E], fp32, kind="ExternalInput")
    WuT = nc.dram_tensor("WuT", [E, E], fp32, kind="ExternalInput")
    WvT = nc.dram_tensor("WvT", [E, E], fp32, kind="ExternalInput")
    blWT = nc.dram_tensor("blWT", [E, E * N], fp32, kind="ExternalInput")
    f3WT = nc.dram_tensor("f3WT", [E, N], fp32, kind="ExternalInput")
    f3WTb = nc.dram_tensor("f3WTb", [E, N], bf16, kind="ExternalInput")
    sb_r = nc.dram_tensor("sb_r", [128, EC], fp32, kind="ExternalInput")
    tb_r = nc.dram_tensor("tb_r", [128, EC], fp32, kind="ExternalInput")
    f2b_r = nc.dram_tensor("f2b_r", [128, EC], fp32, kind="ExternalInput")
    f3b128 = nc.dram_tensor("f3b128", [128, 1], fp32, kind="ExternalInput")
    kxn01 = nc.dram_tensor("kxn01", [128, QUADS], fp32, kind="ExternalInput")
    mask4 = nc.dram_tensor("mask4", [128, 4], fp32, kind="ExternalInput")
    out = nc.dram_tensor("out", [IB, N, L], fp32, kind="ExternalOutput")

    with tile.TileContext(nc) as tc, ExitStack() as ctx:
        consts = ctx.enter_context(tc.tile_pool(name="consts", bufs=1))
        acts = ctx.enter_context(tc.tile_pool(name="acts", bufs=1))

        # ---- DMA in ----
        def load(pool, src, shape, name, dt=fp32):
            t = pool.tile(shape, dt, name=name)
            nc.sync.dma_start(out=t[:], in_=src)
            return t

        xT_sb = [load(consts, xT[128 * c : 128 * (c + 1), :], [128, L], f"xT{c}")
                 for c in range(HC)]
        xTi_sb = [load(consts, xTi[128 * c : 128 * (c + 1), :], [128, IB], f"xTi{c}")
                  for c in range(HC)]
        sWT_sb = [load(consts, sWT[128 * c : 128 * (c + 1), :], [128, E], f"sWT{c}")
                  for c in range(HC)]
        tWT_sb = [load(consts, tWT[128 * c : 128 * (c + 1), :], [128, E], f"tWT{c}")
                  for c in range(HC)]
        WuT_sb = [load(consts, WuT[128 * c : 128 * (c + 1), :], [128, E], f"WuT{c}")
                  for c in range(EC)]
        WvT_sb = [load(consts, WvT[128 * c : 128 * (c + 1), :], [128, E], f"WvT{c}")
                  for c in range(EC)]
        blWT_sb = [load(consts, blWT[128 * c : 128 * (c + 1), :], [128, E * N], f"blWT{c}")
                   for c in range(EC)]
        f3WT_sb = [load(consts, f3WT[128 * c : 128 * (c + 1), :], [128, N], f"f3WT{c}")
                   for c in range(EC)]
        f3WTb_sb = [load(consts, f3WTb[128 * c : 128 * (c + 1), :], [128, N],
                         f"f3WTb{c}", bf16) for c in range(EC)]
        sb_sb = load(consts, sb_r[:], [128, EC], "sb_sb")
        tb_sb = load(consts, tb_r[:], [128, EC], "tb_sb")
        f2b_sb = load(consts, f2b_r[:], [128, EC], "f2b_sb")
        f3b_sb = load(consts, f3b128[:], [128, 1], "f3b_sb")
        kxn01_sb = load(consts, kxn01[:], [128, QUADS], "kxn01_sb")
        mask4_sb = load(consts, mask4[:], [128, 4], "mask4_sb")

        # ---- persistent activations ----
        tT_sb, sT_sb, suT_sb, keyE32_sb = [], [], [], []
        for ec in range(EC):
            tT_sb.append(acts.tile([128, L], fp32, name=f"tT{ec}"))
            sT_sb.append(acts.tile([128, IB], fp32, name=f"sT{ec}"))
            suT_sb.append(acts.tile([128, IB], fp32, name=f"suT{ec}"))
            keyE32_sb.append(acts.tile([128, 32 * IB], fp32, name=f"keyE32_{ec}"))
            nc.gpsimd.memset(keyE32_sb[ec][:], 0.0)
        tvT2 = acts.tile([128, 2 * L], bf16, name="tvT2")      # both e-chunks
        negsu2 = acts.tile([128, 2 * IB], bf16, name="negsu2")  # col 2i+ec
        C4sb = acts.tile([128, QUADS], fp32, name="C4sb")
        CT_sb = acts.tile([128, N], fp32, name="CT_sb")
        kxmC = acts.tile([128, 128], fp32, name="kxmC")
        nc.gpsimd.memset(kxmC[:], 0.0)

        # ---- prep ----
        with tc.tile_pool(name="prep_psum", bufs=3, space="PSUM") as pp:
            for ec in range(EC):
                # tT[e,j] = relu(sum_hc tWT.T @ xT + tb)
                ps_t = pp.tile([128, L], fp32, name="ps_t", tag="ps")
                for hc in range(HC):
                    nc.tensor.matmul(
                        ps_t[:],
                        lhsT=tWT_sb[hc][:, 128 * ec : 128 * (ec + 1)],
                        rhs=xT_sb[hc][:],
                        start=(hc == 0),
                        stop=(hc == HC - 1),
                    )
                nc.scalar.activation(tT_sb[ec][:], ps_t[:], AF.Relu,
                                     bias=tb_sb[:, ec : ec + 1])

                ps_s = pp.tile([128, L], fp32, name="ps_s", tag="ps")
                for hc in range(HC):
                    nc.tensor.matmul(
                        ps_s[:, :IB],
                        lhsT=sWT_sb[hc][:, 128 * ec : 128 * (ec + 1)],
                        rhs=xTi_sb[hc][:],
                        start=(hc == 0),
                        stop=(hc == HC - 1),
                    )
                nc.scalar.activation(sT_sb[ec][:], ps_s[:, :IB], AF.Relu,
                                     bias=sb_sb[:, ec : ec + 1])

            for ec in range(EC):
                # tvT chunk -> tvT2[:, 512*ec:] (bf16)
                ps_tv = pp.tile([128, L], fp32, name="ps_tv", tag="ps")
                for epc in range(EC):
                    nc.tensor.matmul(
                        ps_tv[:],
                        lhsT=WvT_sb[epc][:, 128 * ec : 128 * (ec + 1)],
                        rhs=tT_sb[epc][:],
                        start=(epc == 0),
                        stop=(epc == EC - 1),
                    )
                nc.scalar.copy(tvT2[:, L * ec : L * (ec + 1)], ps_tv[:])

                # suT = WuT.T @ sT + f2b ; negsu2 col-interleaved bf16
                ps_su = pp.tile([128, L], fp32, name="ps_su", tag="ps")
                for epc in range(EC):
                    nc.tensor.matmul(
                        ps_su[:, :IB],
                        lhsT=WuT_sb[epc][:, 128 * ec : 128 * (ec + 1)],
                        rhs=sT_sb[epc][:],
                        start=(epc == 0),
                        stop=(epc == EC - 1),
                    )
                nc.scalar.activation(suT_sb[ec][:], ps_su[:, :IB], AF.Identity,
                                     bias=f2b_sb[:, ec : ec + 1])
                ns2 = negsu2.rearrange("p (i two) -> p i two", two=2)
                nc.vector.tensor_scalar_mul(ns2[:, :, ec], suT_sb[ec][:], -1.0)

            # correction CT[i, n] = sum_e f3WT[e,n] * suT[e,i]  (fp32)
            ps_ct = pp.tile([128, L], fp32, name="ps_ct", tag="ps")
            for ec in range(EC):
                nc.tensor.matmul(
                    ps_ct[:, :N],
                    lhsT=suT_sb[ec][:],
                    rhs=f3WT_sb[ec][:],
                    start=(ec == 0),
                    stop=(ec == EC - 1),
                )
            nc.vector.tensor_copy(out=CT_sb[:], in_=ps_ct[:, :N])
            # kxmC[:, 32k+n] = CT[:, n] * (i%4==k), k=0..2 (lane 3 = ACT, no corr)
            for k in range(3):
                nc.vector.tensor_tensor(
                    out=kxmC[:, 32 * k : 32 * k + N],
                    in0=CT_sb[:],
                    in1=mask4_sb[:, k : k + 1].broadcast_to([128, N]),
                    op=ALU.mult,
                )
            # C4[32k+n, g] = CT[4g+k, n] masked ; + f3b pattern
            ps_c4 = pp.tile([128, L], fp32, name="ps_c4", tag="ps")
            nc.tensor.matmul(ps_c4[:, :QUADS], lhsT=kxmC[:], rhs=kxn01_sb[:],
                             start=True, stop=True)
            nc.vector.tensor_tensor(
                out=C4sb[:], in0=ps_c4[:, :QUADS],
                in1=f3b_sb[:].broadcast_to([128, QUADS]), op=ALU.add)

            # key: keyE32[ec][e, 32*i + n] = sum_e' blW[(128ec+e)*12+n, e'] s[i, e']
            blWT3 = [blWT_sb[c].rearrange("p (e n) -> p e n", n=N) for c in range(EC)]
            key32 = [keyE32_sb[c].rearrange("p (i w) -> p i w", w=32) for c in range(EC)]
            for ec in range(EC):
                for n in range(N):
                    ps_k = pp.tile([128, L], fp32, name="ps_k", tag="ps")
                    for epc in range(EC):
                        nc.tensor.matmul(
                            ps_k[:, :IB],
                            lhsT=blWT3[epc][:, 128 * ec : 128 * (ec + 1), n],
                            rhs=sT_sb[epc][:],
                            start=(epc == 0),
                            stop=(epc == EC - 1),
                        )
                    nc.vector.tensor_copy(out=key32[ec][:, :, n], in_=ps_k[:, :IB])

        # ---- main loop ----
        hp = ctx.enter_context(tc.tile_pool(name="hp", bufs=6))
        outp = ctx.enter_context(tc.tile_pool(name="outp", bufs=3))
        mp = ctx.enter_context(tc.tile_pool(name="main_psum", bufs=4, space="PSUM"))

        for g in range(QUADS):
            ps = mp.tile([128, L], fp32, name="ps")
            # out1: ec0 is M=32 start=True (initializes all 128 rows via
            # zero-padded keyE32), ec1 M=12 accumulate.
            for ec in range(EC):
                for k in range(4):
                    i = 4 * g + k
                    M_ = 32 if ec == 0 else N
                    nc.tensor.matmul(
                        ps[32 * k : 32 * k + M_, :],
                        lhsT=keyE32_sb[ec][:, 32 * i : 32 * i + M_],
                        rhs=tT_sb[ec][:],
                        start=(ec == 0),
                        stop=False,
                        tile_position=(0, 32 * k),
                        skip_group_check=True,
                    )
            # h production
            hs = {}  # (k, ec) -> AP
            for k in range(4):
                i = 4 * g + k
                if k < 3:
                    eng = nc.vector
                    h2 = hp.tile([128, 2 * L], bf16, name="h2", tag="h")
                    in1 = negsu2[:, 2 * i : 2 * i + 2].broadcast_to([128, 2, L])
                    eng.tensor_tensor(out=h2[:].rearrange("p (c f) -> p c f", c=2),
                                      in0=tvT2[:].rearrange("p (c f) -> p c f", c=2),
                                      in1=in1, op=ALU.max)
                    for ec in range(EC):
                        hs[(k, ec)] = h2[:, L * ec : L * (ec + 1)]
                else:
                    for ec in range(EC):
                        ha = hp.tile([128, L], bf16, name="ha", tag="h")
                        nc.scalar.activation(ha[:], tvT2[:, L * ec : L * (ec + 1)],
                                             AF.Relu, bias=suT_sb[ec][:, i : i + 1])
                        hs[(3, ec)] = ha[:]
            # out2 matmuls (bf16)
            for ec in range(EC):
                for k in range(4):
                    nc.tensor.matmul(
                        ps[32 * k : 32 * k + N, :],
                        lhsT=f3WTb_sb[ec][:],
                        rhs=hs[(k, ec)],
                        start=False,
                        stop=(ec == EC - 1),
                        tile_position=(0, 32 * k),
                        skip_group_check=True,
                    )
            # finalize: out = psum + C4[:, g]  (per-partition bias on ACT)
            ob = outp.tile([128, L], fp32, name="ob")
            nc.scalar.activation(ob[:], ps[:], AF.Identity,
                                 bias=C4sb[:, g : g + 1])
            for k in range(4):
                nc.sync.dma_start(out=out[4 * g + k, :, :],
                                  in_=ob[32 * k : 32 * k + N, :])

    nc.compile()
    return nc


def _get_nc():
    if "nc" not in _cache:
        _cache["nc"] = build_nc()
    return _cache["nc"]


def _make_in_maps(inputs):
    import ml_dtypes

    x = np.asarray(inputs["x"], np.float32)
    f32c = lambda a: np.ascontiguousarray(np.asarray(a, np.float32))

    f2W = np.asarray(inputs["f2W"], np.float32)
    f3WT = f32c(inputs["f3W"].T)
    kxn01 = np.zeros((128, QUADS), np.float32)
    mask4 = np.zeros((128, 4), np.float32)
    for i in range(128):
        kxn01[i, i // 4] = 1.0
        mask4[i, i % 4] = 1.0
    shared = {
        "sWT": f32c(inputs["sW"].T),
        "tWT": f32c(inputs["tW"].T),
        "WuT": f32c(f2W[:, :E].T),
        "WvT": f32c(f2W[:, E:].T),
        "blWT": f32c(inputs["blW"].T),
        "f3WT": f3WT,
        "f3WTb": np.ascontiguousarray(f3WT.astype(ml_dtypes.bfloat16)),
        "sb_r": f32c(np.asarray(inputs["sb"], np.float32).reshape(EC, 128).T),
        "tb_r": f32c(np.asarray(inputs["tb"], np.float32).reshape(EC, 128).T),
        "f2b_r": f32c(np.asarray(inputs["f2b"], np.float32).reshape(EC, 128).T),
        "kxn01": kxn01,
        "mask4": mask4,
    }
    f3b128 = np.zeros((128, 1), np.float32)
    for k in range(4):
        f3b128[32 * k : 32 * k + N, 0] = np.asarray(inputs["f3b"], np.float32)
    shared["f3b128"] = f3b128

    in_maps = []
    xT_by_b = [f32c(x[b].T) for b in range(B)]
    for c in range(NCORES):
        b, r = divmod(c, 4)
        m = dict(shared)
        m["xT"] = xT_by_b[b]
        m["xTi"] = f32c(x[b, IB * r : IB * (r + 1), :].T)
        in_maps.append(m)
    return in_maps


def _gather(results):
    full = np.empty((B, L, N, L), np.float32)
    for c in range(NCORES):
        b, r = divmod(c, 4)
        full[b, IB * r : IB * (r + 1)] = results[c]["out"]
    return full


def kernel(x, sW, sb, tW, tb, f2W, f2b, f3W, f3b, blW):
    from concourse.bass_utils import run_bass_kernel_spmd

    in_maps = _make_in_maps(dict(
        x=x, sW=sW, sb=sb, tW=tW, tb=tb, f2W=f2W, f2b=f2b,
        f3W=f3W, f3b=f3b, blW=blW,
    ))
    nc = _get_nc()
    res = run_bass_kernel_spmd(nc, in_maps, core_ids=list(range(NCORES)))
    return _gather(res.results)


# revision 14
# speedup vs baseline: 3.7634x; 1.2089x over previous
"""BiAffine layer kernel for 8 Trainium2 NeuronCores.

Reference computation (per batch b):
  s = relu(x @ sW.T + sb)                  [L, E]
  t = relu(x @ tW.T + tb)                  [L, E]
  key = (s @ blW.T).reshape(L, E, N)
  out1[i, n, l] = sum_e key[i, e, n] * t[l, e]
  su = s @ Wu.T ; tv = t @ Wv.T            (Wu, Wv = f2W[:, :E], f2W[:, E:])
  h[i, j, :] = relu(su[i] + tv[j] + f2b)
  out2[i, n, j] = sum_e h[i, j, e] * f3W[n, e] + f3b[n]
  out = out1 + out2                        [L, N, L]

Sharding: 8 cores = 2 batches x 4 blocks of 128 source positions (i).

Main loop: i in quads; one PSUM bank [128, 512] holds 4 outputs at
32-aligned partition offsets (rows 32k+n), M=12/32 matmuls in 4 distinct
PE column groups (tile concurrency).

h production per lane k = i%4:
  k=0,1 -> DVE, k=2 -> GPSIMD: h' = max(tv, -su') via one bf16
    tensor_tensor with a free-broadcast AP ([128,1024] = both e-chunks).
    h' = relu(tv+su') - su', so the matmul result is short by
    sum_e f3W[n,e]*su'[e,i]; that rank-1 term C is precomputed once
    (PE) and folded into the final copy's per-partition bias.
  k=3 -> ACT: true h = relu(tv + su') via activation bias (native fast).
Final copy: ACT activation(Identity, bias = C4[:, g] + f3b pattern).
"""

import sys

sys.path.insert(0, "/opt/trn_rl_repo")

import numpy as np

B, L, H, E, N = 2, 512, 768, 256, 12
EC = E // 128  # 2 e-chunks
HC = H // 128  # 6 h-chunks
IB = L // 4  # 128 i's per core
NCORES = 8
QUADS = IB // 4  # 32

_cache = {}


def build_nc():
    import concourse.bass as bass
    import concourse.tile as tile
    from concourse import bacc, mybir
    from contextlib import ExitStack

    fp32 = mybir.dt.float32
    bf16 = mybir.dt.bfloat16
    AF = mybir.ActivationFunctionType
    ALU = mybir.AluOpType

    nc = bacc.Bacc("TRN2")

    # ---- I/O ----
    xT = nc.dram_tensor("xT", [H, L], fp32, kind="ExternalInput")
    xTi = nc.dram_tensor("xTi", [H, IB], fp32, kind="ExternalInput")
    sWT = nc.dram_tensor("sWT", [H, E], fp32, kind="ExternalInput")
    tWT = nc.dram_tensor("tWT", [H, E], fp32, kind="ExternalInput")
    WuT = nc.dram_tensor("WuT", [E, E], fp32, kind="ExternalInput")
    WvT = nc.dram_tensor("WvT", [E, E], fp32, kind="ExternalInput")
    blWT = nc.dram_tensor("blWT", [E, E * N], fp32, kind="ExternalInput")
    f3WT = nc.dram_tensor("f3WT", [E, N], fp32, kind="ExternalInput")
    f3WTb = nc.dram_tensor("f3WTb", [E, N], bf16, kind="ExternalInput")
    sb_r = nc.dram_tensor("sb_r", [128, EC], fp32, kind="ExternalInput")
    tb_r = nc.dram_tensor("tb_r", [128, EC], fp32, kind="ExternalInput")
    f2b_r = nc.dram_tensor("f2b_r", [128, EC], fp32, kind="ExternalInput")
    f3b128 = nc.dram_tensor("f3b128", [128, 1], fp32, kind="ExternalInput")
    kxn01 = nc.dram_tensor("kxn01", [128, QUADS], fp32, kind="ExternalInput")
    mask4 = nc.dram_tensor("mask4", [128, 4], fp32, kind="ExternalInput")
    out = nc.dram_tensor("out", [IB, N, L], fp32, kind="ExternalOutput")

    with tile.TileContext(nc) as tc, ExitStack() as ctx:
        consts = ctx.enter_context(tc.tile_pool(name="consts", bufs=1))
        acts = ctx.enter_context(tc.tile_pool(name="acts", bufs=1))

        # ---- DMA in ----
        def load(pool, src, shape, name, dt=fp32):
            t = pool.tile(shape, dt, name=name)
            nc.sync.dma_start(out=t[:], in_=src)
            return t

        xT_sb = [load(consts, xT[128 * c : 128 * (c + 1), :], [128, L], f"xT{c}")
                 for c in range(HC)]
        xTi_sb = [load(consts, xTi[128 * c : 128 * (c + 1), :], [128, IB], f"xTi{c}")
                  for c in range(HC)]
        sWT_sb = [load(consts, sWT[128 * c : 128 * (c + 1), :], [128, E], f"sWT{c}")
                  for c in range(HC)]
        tWT_sb = [load(consts, tWT[128 * c : 128 * (c + 1), :], [128, E], f"tWT{c}")
                  for c in range(HC)]
        WuT_sb = [load(consts, WuT[128 * c : 128 * (c + 1), :], [128, E], f"WuT{c}")
                  for c in range(EC)]
        WvT_sb = [load(consts, WvT[128 * c : 128 * (c + 1), :], [128, E], f"WvT{c}")
                  for c in range(EC)]
        blWT_sb = [load(consts, blWT[128 * c : 128 * (c + 1), :], [128, E * N], f"blWT{c}")
                   for c in range(EC)]
        f3WT_sb = [load(consts, f3WT[128 * c : 128 * (c + 1), :], [128, N], f"f3WT{c}")
                   for c in range(EC)]
        f3WTb_sb = [load(consts, f3WTb[128 * c : 128 * (c + 1), :], [128, N],
                         f"f3WTb{c}", bf16) for c in range(EC)]
        sb_sb = load(consts, sb_r[:], [128, EC], "sb_sb")
        tb_sb = load(consts, tb_r[:], [128, EC], "tb_sb")
        f2b_sb = load(consts, f2b_r[:], [128, EC], "f2b_sb")
        f3b_sb = load(consts, f3b128[:], [128, 1], "f3b_sb")
        kxn01_sb = load(consts, kxn01[:], [128, QUADS], "kxn01_sb")
        mask4_sb = load(consts, mask4[:], [128, 4], "mask4_sb")

        # ---- persistent activations ----
        tT_sb, sT_sb, suT_sb, keyE32_sb = [], [], [], []
        for ec in range(EC):
            tT_sb.append(acts.tile([128, L], fp32, name=f"tT{ec}"))
            sT_sb.append(acts.tile([128, IB], fp32, name=f"sT{ec}"))
            suT_sb.append(acts.tile([128, IB], fp32, name=f"suT{ec}"))
            keyE32_sb.append(acts.tile([128, 32 * IB], fp32, name=f"keyE32_{ec}"))
            nc.gpsimd.memset(keyE32_sb[ec][:], 0.0)
        tvT2c = acts.tile([128, 2 * L], bf16, name="tvT2c")  # cols 512*ec+j
        tvT2i = acts.tile([128, 2 * L], bf16, name="tvT2i")  # cols 2*j+ec
        negsu2 = acts.tile([128, 2 * IB], bf16, name="negsu2")  # col 2i+ec
        C4sb = acts.tile([128, QUADS], fp32, name="C4sb")
        CT_sb = acts.tile([128, N], fp32, name="CT_sb")
        kxmC = acts.tile([128, 128], fp32, name="kxmC")
        nc.gpsimd.memset(kxmC[:], 0.0)

        # ---- prep ----
        with tc.tile_pool(name="prep_psum", bufs=3, space="PSUM") as pp:
            for ec in range(EC):
                # tT[e,j] = relu(sum_hc tWT.T @ xT + tb)
                ps_t = pp.tile([128, L], fp32, name="ps_t", tag="ps")
                for hc in range(HC):
                    nc.tensor.matmul(
                        ps_t[:],
                        lhsT=tWT_sb[hc][:, 128 * ec : 128 * (ec + 1)],
                        rhs=xT_sb[hc][:],
                        start=(hc == 0),
                        stop=(hc == HC - 1),
                    )
                nc.scalar.activation(tT_sb[ec][:], ps_t[:], AF.Relu,
                                     bias=tb_sb[:, ec : ec + 1])

                ps_s = pp.tile([128, L], fp32, name="ps_s", tag="ps")
                for hc in range(HC):
                    nc.tensor.matmul(
                        ps_s[:, :IB],
                        lhsT=sWT_sb[hc][:, 128 * ec : 128 * (ec + 1)],
                        rhs=xTi_sb[hc][:],
                        start=(hc == 0),
                        stop=(hc == HC - 1),
                    )
                nc.scalar.activation(sT_sb[ec][:], ps_s[:, :IB], AF.Relu,
                                     bias=sb_sb[:, ec : ec + 1])

            for ec in range(EC):
                # tvT chunk -> tvT2[:, 512*ec:] (bf16)
                ps_tv = pp.tile([128, L], fp32, name="ps_tv", tag="ps")
                for epc in range(EC):
                    nc.tensor.matmul(
                        ps_tv[:],
                        lhsT=WvT_sb[epc][:, 128 * ec : 128 * (ec + 1)],
                        rhs=tT_sb[epc][:],
                        start=(epc == 0),
                        stop=(epc == EC - 1),
                    )
                nc.scalar.copy(tvT2c[:, L * ec : L * (ec + 1)], ps_tv[:])
                tv_i = tvT2i.rearrange("p (f c) -> p f c", c=2)
                nc.vector.tensor_copy(out=tv_i[:, :, ec], in_=ps_tv[:])

                # suT = WuT.T @ sT + f2b ; negsu2 col-interleaved bf16
                ps_su = pp.tile([128, L], fp32, name="ps_su", tag="ps")
                for epc in range(EC):
                    nc.tensor.matmul(
                        ps_su[:, :IB],
                        lhsT=WuT_sb[epc][:, 128 * ec : 128 * (ec + 1)],
                        rhs=sT_sb[epc][:],
                        start=(epc == 0),
                        stop=(epc == EC - 1),
                    )
                nc.scalar.activation(suT_sb[ec][:], ps_su[:, :IB], AF.Identity,
                                     bias=f2b_sb[:, ec : ec + 1])
                ns2 = negsu2.rearrange("p (i two) -> p i two", two=2)
                nc.vector.tensor_scalar_mul(ns2[:, :, ec], suT_sb[ec][:], -1.0)

            # correction CT[i, n] = sum_e f3WT[e,n] * suT[e,i]  (fp32)
            ps_ct = pp.tile([128, L], fp32, name="ps_ct", tag="ps")
            for ec in range(EC):
                nc.tensor.matmul(
                    ps_ct[:, :N],
                    lhsT=suT_sb[ec][:],
                    rhs=f3WT_sb[ec][:],
                    start=(ec == 0),
                    stop=(ec == EC - 1),
                )
            nc.vector.tensor_copy(out=CT_sb[:], in_=ps_ct[:, :N])
            # kxmC[:, 32k+n] = CT[:, n] * (i%4==k), k=0..2 (lane 3 = ACT, no corr)
            for k in range(3):
                nc.vector.tensor_tensor(
                    out=kxmC[:, 32 * k : 32 * k + N],
                    in0=CT_sb[:],
                    in1=mask4_sb[:, k : k + 1].broadcast_to([128, N]),
                    op=ALU.mult,
                )
            # C4[32k+n, g] = CT[4g+k, n] masked ; + f3b pattern
            ps_c4 = pp.tile([128, L], fp32, name="ps_c4", tag="ps")
            nc.tensor.matmul(ps_c4[:, :QUADS], lhsT=kxmC[:], rhs=kxn01_sb[:],
                             start=True, stop=True)
            nc.vector.tensor_tensor(
                out=C4sb[:], in0=ps_c4[:, :QUADS],
                in1=f3b_sb[:].broadcast_to([128, QUADS]), op=ALU.add)

            # key: keyE32[ec][e, 32*i + n] = sum_e' blW[(128ec+e)*12+n, e'] s[i, e']
            blWT3 = [blWT_sb[c].rearrange("p (e n) -> p e n", n=N) for c in range(EC)]
            key32 = [keyE32_sb[c].rearrange("p (i w) -> p i w", w=32) for c in range(EC)]
            for ec in range(EC):
                for n in range(N):
                    ps_k = pp.tile([128, L], fp32, name="ps_k", tag="ps")
                    for epc in range(EC):
                        nc.tensor.matmul(
                            ps_k[:, :IB],
                            lhsT=blWT3[epc][:, 128 * ec : 128 * (ec + 1), n],
                            rhs=sT_sb[epc][:],
                            start=(epc == 0),
                            stop=(epc == EC - 1),
                        )
                    nc.vector.tensor_copy(out=key32[ec][:, :, n], in_=ps_k[:, :IB])

        # ---- main loop ----
        hp = ctx.enter_context(tc.tile_pool(name="hp", bufs=16))
        outp = ctx.enter_context(tc.tile_pool(name="outp", bufs=4))
        mp = ctx.enter_context(tc.tile_pool(name="main_psum", bufs=6, space="PSUM"))

        for g in range(QUADS):
            ps = mp.tile([128, L], fp32, name="ps")
            # out1: ec0 is M=32 start=True (initializes all 128 rows via
            # zero-padded keyE32), ec1 M=12 accumulate.
            for ec in range(EC):
                for k in range(4):
                    i = 4 * g + k
                    M_ = 32 if ec == 0 else N
                    nc.tensor.matmul(
                        ps[32 * k : 32 * k + M_, :],
                        lhsT=keyE32_sb[ec][:, 32 * i : 32 * i + M_],
                        rhs=tT_sb[ec][:],
                        start=(ec == 0),
                        stop=False,
                        tile_position=(0, 32 * k),
                        skip_group_check=True,
                    )
            # h production
            hs = {}  # (k, ec) -> AP
            for k in range(4):
                i = 4 * g + k
                if k < 3:
                    h2 = hp.tile([128, 2 * L], bf16, name="h2", tag="h")
                    in1 = negsu2[:, 2 * i : 2 * i + 2].unsqueeze(1)\
                        .broadcast_to([128, L, 2])
                    nc.vector.tensor_tensor(
                        out=h2[:].rearrange("p (f c) -> p f c", c=2),
                        in0=tvT2i[:].rearrange("p (f c) -> p f c", c=2),
                        in1=in1, op=ALU.max)
                    h2v = h2.rearrange("p (f c) -> p f c", c=2)
                    for ec in range(EC):
                        hs[(k, ec)] = h2v[:, :, ec]
                else:
                    for ec in range(EC):
                        ha = hp.tile([128, L], bf16, name="ha", tag="h")
                        nc.scalar.activation(ha[:], tvT2c[:, L * ec : L * (ec + 1)],
                                             AF.Relu, bias=suT_sb[ec][:, i : i + 1])
                        hs[(3, ec)] = ha[:]
            # out2 matmuls (bf16)
            for ec in range(EC):
                for k in range(4):
                    nc.tensor.matmul(
                        ps[32 * k : 32 * k + N, :],
                        lhsT=f3WTb_sb[ec][:],
                        rhs=hs[(k, ec)],
                        start=False,
                        stop=(ec == EC - 1),
                        tile_position=(0, 32 * k),
                        skip_group_check=True,
                    )
            # finalize: out = psum + C4[:, g]  (per-partition bias on ACT)
            ob = outp.tile([128, L], fp32, name="ob")
            nc.scalar.activation(ob[:], ps[:], AF.Identity,
                                 bias=C4sb[:, g : g + 1])
            for k in range(4):
                dma_eng = nc.sync if k % 2 == 0 else nc.gpsimd
                dma_eng.dma_start(out=out[4 * g + k, :, :],
                                  in_=ob[32 * k : 32 * k + N, :])

    nc.compile()
    return nc


def _get_nc():
    if "nc" not in _cache:
        _cache["nc"] = build_nc()
    return _cache["nc"]


def _make_in_maps(inputs):
    import ml_dtypes

    x = np.asarray(inputs["x"], np.float32)
    f32c = lambda a: np.ascontiguousarray(np.asarray(a, np.float32))

    f2W = np.asarray(inputs["f2W"], np.float32)
    f3WT = f32c(inputs["f3W"].T)
    kxn01 = np.zeros((128, QUADS), np.float32)
    mask4 = np.zeros((128, 4), np.float32)
    for i in range(128):
        kxn01[i, i // 4] = 1.0
        mask4[i, i % 4] = 1.0
    shared = {
        "sWT": f32c(inputs["sW"].T),
        "tWT": f32c(inputs["tW"].T),
        "WuT": f32c(f2W[:, :E].T),
        "WvT": f32c(f2W[:, E:].T),
        "blWT": f32c(inputs["blW"].T),
        "f3WT": f3WT,
        "f3WTb": np.ascontiguousarray(f3WT.astype(ml_dtypes.bfloat16)),
        "sb_r": f32c(np.asarray(inputs["sb"], np.float32).reshape(EC, 128).T),
        "tb_r": f32c(np.asarray(inputs["tb"], np.float32).reshape(EC, 128).T),
        "f2b_r": f32c(np.asarray(inputs["f2b"], np.float32).reshape(EC, 128).T),
        "kxn01": kxn01,
        "mask4": mask4,
    }
    f3b128 = np.zeros((128, 1), np.float32)
    for k in range(4):
        f3b128[32 * k : 32 * k + N, 0] = np.asarray(inputs["f3b"], np.float32)
    shared["f3b128"] = f3b128

    in_maps = []
    xT_by_b = [f32c(x[b].T) for b in range(B)]
    for c in range(NCORES):
        b, r = divmod(c, 4)
        m = dict(shared)
        m["xT"] = xT_by_b[b]
        m["xTi"] = f32c(x[b, IB * r : IB * (r + 1), :].T)
        in_maps.append(m)
    return in_maps


def _gather(results):
    full = np.empty((B, L, N, L), np.float32)
    for c in range(NCORES):
        b, r = divmod(c, 4)
        full[b, IB * r : IB * (r + 1)] = results[c]["out"]
    return full


def kernel(x, sW, sb, tW, tb, f2W, f2b, f3W, f3b, blW):
    from concourse.bass_utils import run_bass_kernel_spmd

    in_maps = _make_in_maps(dict(
        x=x, sW=sW, sb=sb, tW=tW, tb=tb, f2W=f2W, f2b=f2b,
        f3W=f3W, f3b=f3b, blW=blW,
    ))
    nc = _get_nc()
    res = run_bass_kernel_spmd(nc, in_maps, core_ids=list(range(NCORES)))
    return _gather(res.results)


# revision 17
# speedup vs baseline: 4.1393x; 1.0999x over previous
"""BiAffine layer kernel for 8 Trainium2 NeuronCores.

Reference computation (per batch b):
  s = relu(x @ sW.T + sb)                  [L, E]
  t = relu(x @ tW.T + tb)                  [L, E]
  key = (s @ blW.T).reshape(L, E, N)
  out1[i, n, l] = sum_e key[i, e, n] * t[l, e]
  su = s @ Wu.T ; tv = t @ Wv.T            (Wu, Wv = f2W[:, :E], f2W[:, E:])
  h[i, j, :] = relu(su[i] + tv[j] + f2b)
  out2[i, n, j] = sum_e h[i, j, e] * f3W[n, e] + f3b[n]
  out = out1 + out2                        [L, N, L]

Sharding: 8 cores = 2 batches x 4 blocks of 128 source positions (i).

Main loop: i in quads; one PSUM bank [128, 512] holds 4 outputs at
32-aligned partition offsets (rows 32k+n), M=12/32 matmuls in 4 distinct
PE column groups (tile concurrency).

h production per lane k = i%4:
  k=0,1 -> DVE, k=2 -> GPSIMD: h' = max(tv, -su') via one bf16
    tensor_tensor with a free-broadcast AP ([128,1024] = both e-chunks).
    h' = relu(tv+su') - su', so the matmul result is short by
    sum_e f3W[n,e]*su'[e,i]; that rank-1 term C is precomputed once
    (PE) and folded into the final copy's per-partition bias.
  k=3 -> ACT: true h = relu(tv + su') via activation bias (native fast).
Final copy: ACT activation(Identity, bias = C4[:, g] + f3b pattern).
"""

import sys

sys.path.insert(0, "/opt/trn_rl_repo")

import numpy as np

B, L, H, E, N = 2, 512, 768, 256, 12
EC = E // 128  # 2 e-chunks
HC = H // 128  # 6 h-chunks
IB = L // 4  # 128 i's per core
NCORES = 8
QUADS = IB // 4  # 32

_cache = {}


def build_nc():
    import concourse.bass as bass
    import concourse.tile as tile
    from concourse import bacc, mybir
    from contextlib import ExitStack

    fp32 = mybir.dt.float32
    bf16 = mybir.dt.float16  # 16-bit compute dtype (fp16: 10 mantissa bits)
    AF = mybir.ActivationFunctionType
    ALU = mybir.AluOpType

    nc = bacc.Bacc("TRN2")

    # ---- I/O ----
    xT = nc.dram_tensor("xT", [H, L], fp32, kind="ExternalInput")
    xTi = nc.dram_tensor("xTi", [H, IB], fp32, kind="ExternalInput")
    sWT = nc.dram_tensor("sWT", [H, E], fp32, kind="ExternalInput")
    tWT = nc.dram_tensor("tWT", [H, E], fp32, kind="ExternalInput")
    WuT = nc.dram_tensor("WuT", [E, E], fp32, kind="ExternalInput")
    WvT = nc.dram_tensor("WvT", [E, E], fp32, kind="ExternalInput")
    blWT = nc.dram_tensor("blWT", [E, E * N], fp32, kind="ExternalInput")
    f3WT = nc.dram_tensor("f3WT", [E, N], fp32, kind="ExternalInput")
    f3WTb = nc.dram_tensor("f3WTb", [E, N], bf16, kind="ExternalInput")
    sb_r = nc.dram_tensor("sb_r", [128, EC], fp32, kind="ExternalInput")
    tb_r = nc.dram_tensor("tb_r", [128, EC], fp32, kind="ExternalInput")
    f2b_r = nc.dram_tensor("f2b_r", [128, EC], fp32, kind="ExternalInput")
    f3b128 = nc.dram_tensor("f3b128", [128, 1], fp32, kind="ExternalInput")
    kxn01 = nc.dram_tensor("kxn01", [128, QUADS], fp32, kind="ExternalInput")
    mask4 = nc.dram_tensor("mask4", [128, 4], fp32, kind="ExternalInput")
    out = nc.dram_tensor("out", [IB, N, L], fp32, kind="ExternalOutput")

    with tile.TileContext(nc) as tc, ExitStack() as ctx:
        consts = ctx.enter_context(tc.tile_pool(name="consts", bufs=1))
        acts = ctx.enter_context(tc.tile_pool(name="acts", bufs=1))

        # ---- DMA in ----
        def load(pool, src, shape, name, dt=fp32, eng=None):
            t = pool.tile(shape, dt, name=name)
            (eng or nc.sync).dma_start(out=t[:], in_=src)
            return t

        xT_sb = [load(consts, xT[128 * c : 128 * (c + 1), :], [128, L], f"xT{c}")
                 for c in range(HC)]
        xTi_sb = [load(consts, xTi[128 * c : 128 * (c + 1), :], [128, IB], f"xTi{c}",
                       eng=nc.gpsimd) for c in range(HC)]
        sWT_sb = [load(consts, sWT[128 * c : 128 * (c + 1), :], [128, E], f"sWT{c}",
                       eng=nc.gpsimd) for c in range(HC)]
        tWT_sb = [load(consts, tWT[128 * c : 128 * (c + 1), :], [128, E], f"tWT{c}")
                  for c in range(HC)]
        WuT_sb = [load(consts, WuT[128 * c : 128 * (c + 1), :], [128, E], f"WuT{c}",
                       eng=nc.scalar) for c in range(EC)]
        WvT_sb = [load(consts, WvT[128 * c : 128 * (c + 1), :], [128, E], f"WvT{c}",
                       eng=nc.scalar) for c in range(EC)]
        blWT_sb = [load(consts, blWT[128 * c : 128 * (c + 1), :], [128, E * N],
                        f"blWT{c}", eng=nc.scalar) for c in range(EC)]
        f3WT_sb = [load(consts, f3WT[128 * c : 128 * (c + 1), :], [128, N], f"f3WT{c}",
                        eng=nc.gpsimd) for c in range(EC)]
        f3WTb_sb = [load(consts, f3WTb[128 * c : 128 * (c + 1), :], [128, N],
                         f"f3WTb{c}", bf16) for c in range(EC)]
        sb_sb = load(consts, sb_r[:], [128, EC], "sb_sb")
        tb_sb = load(consts, tb_r[:], [128, EC], "tb_sb")
        f2b_sb = load(consts, f2b_r[:], [128, EC], "f2b_sb")
        f3b_sb = load(consts, f3b128[:], [128, 1], "f3b_sb")
        kxn01_sb = load(consts, kxn01[:], [128, QUADS], "kxn01_sb")
        mask4_sb = load(consts, mask4[:], [128, 4], "mask4_sb")

        # ---- persistent activations ----
        tT_sb, sT_sb, suT_sb, keyE32_sb = [], [], [], []
        for ec in range(EC):
            tT_sb.append(acts.tile([128, L], fp32, name=f"tT{ec}"))
            sT_sb.append(acts.tile([128, IB], fp32, name=f"sT{ec}"))
            suT_sb.append(acts.tile([128, IB], fp32, name=f"suT{ec}"))
            keyE32_sb.append(acts.tile([128, 32 * IB], bf16, name=f"keyE32_{ec}"))
            nc.gpsimd.memset(keyE32_sb[ec][:], 0.0)
        tTb2 = acts.tile([128, 2 * L], bf16, name="tTb2")    # bf16 tT, cols 512*ec+j
        tvT2c = acts.tile([128, 2 * L], bf16, name="tvT2c")  # cols 512*ec+j
        tvT2i = acts.tile([128, 2 * L], bf16, name="tvT2i")  # cols 2*j+ec
        negsu2 = acts.tile([128, 2 * IB], bf16, name="negsu2")  # col 2i+ec
        C4sb = acts.tile([128, QUADS], fp32, name="C4sb")
        CT_sb = acts.tile([128, N], fp32, name="CT_sb")
        kxmC = acts.tile([128, 128], fp32, name="kxmC")
        nc.gpsimd.memset(kxmC[:], 0.0)

        # ---- prep ----
        with tc.tile_pool(name="prep_psum", bufs=3, space="PSUM") as pp:
            for ec in range(EC):
                # tT[e,j] = relu(sum_hc tWT.T @ xT + tb)
                ps_t = pp.tile([128, L], fp32, name="ps_t", tag="ps")
                for hc in range(HC):
                    nc.tensor.matmul(
                        ps_t[:],
                        lhsT=tWT_sb[hc][:, 128 * ec : 128 * (ec + 1)],
                        rhs=xT_sb[hc][:],
                        start=(hc == 0),
                        stop=(hc == HC - 1),
                    )
                nc.scalar.activation(tT_sb[ec][:], ps_t[:], AF.Relu,
                                     bias=tb_sb[:, ec : ec + 1])
                nc.vector.tensor_copy(out=tTb2[:, L * ec : L * (ec + 1)],
                                      in_=tT_sb[ec][:])

                ps_s = pp.tile([128, L], fp32, name="ps_s", tag="ps")
                for hc in range(HC):
                    nc.tensor.matmul(
                        ps_s[:, :IB],
                        lhsT=sWT_sb[hc][:, 128 * ec : 128 * (ec + 1)],
                        rhs=xTi_sb[hc][:],
                        start=(hc == 0),
                        stop=(hc == HC - 1),
                    )
                nc.scalar.activation(sT_sb[ec][:], ps_s[:, :IB], AF.Relu,
                                     bias=sb_sb[:, ec : ec + 1])

            for ec in range(EC):
                # tvT chunk -> tvT2[:, 512*ec:] (bf16)
                ps_tv = pp.tile([128, L], fp32, name="ps_tv", tag="ps")
                for epc in range(EC):
                    nc.tensor.matmul(
                        ps_tv[:],
                        lhsT=WvT_sb[epc][:, 128 * ec : 128 * (ec + 1)],
                        rhs=tT_sb[epc][:],
                        start=(epc == 0),
                        stop=(epc == EC - 1),
                    )
                nc.scalar.copy(tvT2c[:, L * ec : L * (ec + 1)], ps_tv[:])
                tv_i = tvT2i.rearrange("p (f c) -> p f c", c=2)
                nc.vector.tensor_copy(out=tv_i[:, :, ec], in_=ps_tv[:])

                # suT = WuT.T @ sT + f2b ; negsu2 col-interleaved bf16
                ps_su = pp.tile([128, L], fp32, name="ps_su", tag="ps")
                for epc in range(EC):
                    nc.tensor.matmul(
                        ps_su[:, :IB],
                        lhsT=WuT_sb[epc][:, 128 * ec : 128 * (ec + 1)],
                        rhs=sT_sb[epc][:],
                        start=(epc == 0),
                        stop=(epc == EC - 1),
                    )
                nc.scalar.activation(suT_sb[ec][:], ps_su[:, :IB], AF.Identity,
                                     bias=f2b_sb[:, ec : ec + 1])
                ns2 = negsu2.rearrange("p (i two) -> p i two", two=2)
                nc.vector.tensor_scalar_mul(ns2[:, :, ec], suT_sb[ec][:], -1.0)

            # correction CT[i, n] = sum_e f3WT[e,n] * suT[e,i]  (fp32)
            ps_ct = pp.tile([128, L], fp32, name="ps_ct", tag="ps")
            for ec in range(EC):
                nc.tensor.matmul(
                    ps_ct[:, :N],
                    lhsT=suT_sb[ec][:],
                    rhs=f3WT_sb[ec][:],
                    start=(ec == 0),
                    stop=(ec == EC - 1),
                )
            nc.vector.tensor_copy(out=CT_sb[:], in_=ps_ct[:, :N])
            # kxmC[:, 32k+n] = CT[:, n] * (i%4==k), k=0..2 (lane 3 = ACT, no corr)
            for k in range(3):
                nc.vector.tensor_tensor(
                    out=kxmC[:, 32 * k : 32 * k + N],
                    in0=CT_sb[:],
                    in1=mask4_sb[:, k : k + 1].broadcast_to([128, N]),
                    op=ALU.mult,
                )
            # C4[32k+n, g] = CT[4g+k, n] masked ; + f3b pattern
            ps_c4 = pp.tile([128, L], fp32, name="ps_c4", tag="ps")
            nc.tensor.matmul(ps_c4[:, :QUADS], lhsT=kxmC[:], rhs=kxn01_sb[:],
                             start=True, stop=True)
            nc.vector.tensor_tensor(
                out=C4sb[:], in0=ps_c4[:, :QUADS],
                in1=f3b_sb[:].broadcast_to([128, QUADS]), op=ALU.add)

            # key: keyE32[ec][e, 32*i + n] = sum_e' blW[(128ec+e)*12+n, e'] s[i, e']
            blWT3 = [blWT_sb[c].rearrange("p (e n) -> p e n", n=N) for c in range(EC)]
            key32 = [keyE32_sb[c].rearrange("p (i w) -> p i w", w=32) for c in range(EC)]
            for ec in range(EC):
                for n in range(N):
                    ps_k = pp.tile([128, L], fp32, name="ps_k", tag="ps")
                    for epc in range(EC):
                        nc.tensor.matmul(
                            ps_k[:, :IB],
                            lhsT=blWT3[epc][:, 128 * ec : 128 * (ec + 1), n],
                            rhs=sT_sb[epc][:],
                            start=(epc == 0),
                            stop=(epc == EC - 1),
                        )
                    nc.vector.tensor_copy(out=key32[ec][:, :, n], in_=ps_k[:, :IB])

        # ---- main loop ----
        hp = ctx.enter_context(tc.tile_pool(name="hp", bufs=16))
        outp = ctx.enter_context(tc.tile_pool(name="outp", bufs=4))
        mp = ctx.enter_context(tc.tile_pool(name="main_psum", bufs=6, space="PSUM"))

        for g in range(QUADS):
            ps = mp.tile([128, L], fp32, name="ps")
            # out1: ec0 is M=32 start=True (initializes all 128 rows via
            # zero-padded keyE32), ec1 M=12 accumulate.
            for ec in range(EC):
                for k in range(4):
                    i = 4 * g + k
                    M_ = 32 if ec == 0 else N
                    nc.tensor.matmul(
                        ps[32 * k : 32 * k + M_, :],
                        lhsT=keyE32_sb[ec][:, 32 * i : 32 * i + M_],
                        rhs=tTb2[:, L * ec : L * (ec + 1)],
                        start=(ec == 0),
                        stop=False,
                        tile_position=(0, 32 * k),
                        skip_group_check=True,
                    )
            # h production
            hs = {}  # (k, ec) -> AP
            for k in range(4):
                i = 4 * g + k
                if k < 3:
                    h2 = hp.tile([128, 2 * L], bf16, name="h2", tag="h")
                    in1 = negsu2[:, 2 * i : 2 * i + 2].unsqueeze(1)\
                        .broadcast_to([128, L, 2])
                    nc.vector.tensor_tensor(
                        out=h2[:].rearrange("p (f c) -> p f c", c=2),
                        in0=tvT2i[:].rearrange("p (f c) -> p f c", c=2),
                        in1=in1, op=ALU.max)
                    h2v = h2.rearrange("p (f c) -> p f c", c=2)
                    for ec in range(EC):
                        hs[(k, ec)] = h2v[:, :, ec]
                else:
                    for ec in range(EC):
                        ha = hp.tile([128, L], bf16, name="ha", tag="h")
                        nc.scalar.activation(ha[:], tvT2c[:, L * ec : L * (ec + 1)],
                                             AF.Relu, bias=suT_sb[ec][:, i : i + 1])
                        hs[(3, ec)] = ha[:]
            # out2 matmuls (bf16)
            for ec in range(EC):
                for k in range(4):
                    nc.tensor.matmul(
                        ps[32 * k : 32 * k + N, :],
                        lhsT=f3WTb_sb[ec][:],
                        rhs=hs[(k, ec)],
                        start=False,
                        stop=(ec == EC - 1),
                        tile_position=(0, 32 * k),
                        skip_group_check=True,
                    )
            # finalize: out = psum + C4[:, g]  (per-partition bias on ACT)
            ob = outp.tile([128, L], fp32, name="ob")
            nc.scalar.activation(ob[:], ps[:], AF.Identity,
                                 bias=C4sb[:, g : g + 1])
            for k in range(4):
                dma_eng = nc.sync if k % 2 == 0 else nc.gpsimd
                dma_eng.dma_start(out=out[4 * g + k, :, :],
                                  in_=ob[32 * k : 32 * k + N, :])

    nc.compile()
    return nc


def _get_nc():
    if "nc" not in _cache:
        _cache["nc"] = build_nc()
    return _cache["nc"]


def _make_in_maps(inputs):
    import ml_dtypes

    x = np.asarray(inputs["x"], np.float32)
    f32c = lambda a: np.ascontiguousarray(np.asarray(a, np.float32))

    f2W = np.asarray(inputs["f2W"], np.float32)
    f3WT = f32c(inputs["f3W"].T)
    kxn01 = np.zeros((128, QUADS), np.float32)
    mask4 = np.zeros((128, 4), np.float32)
    for i in range(128):
        kxn01[i, i // 4] = 1.0
        mask4[i, i % 4] = 1.0
    shared = {
        "sWT": f32c(inputs["sW"].T),
        "tWT": f32c(inputs["tW"].T),
        "WuT": f32c(f2W[:, :E].T),
        "WvT": f32c(f2W[:, E:].T),
        "blWT": f32c(inputs["blW"].T),
        "f3WT": f3WT,
        "f3WTb": np.ascontiguousarray(f3WT.astype(np.float16)),
        "sb_r": f32c(np.asarray(inputs["sb"], np.float32).reshape(EC, 128).T),
        "tb_r": f32c(np.asarray(inputs["tb"], np.float32).reshape(EC, 128).T),
        "f2b_r": f32c(np.asarray(inputs["f2b"], np.float32).reshape(EC, 128).T),
        "kxn01": kxn01,
        "mask4": mask4,
    }
    f3b128 = np.zeros((128, 1), np.float32)
    for k in range(4):
        f3b128[32 * k : 32 * k + N, 0] = np.asarray(inputs["f3b"], np.float32)
    shared["f3b128"] = f3b128

    in_maps = []
    xT_by_b = [f32c(x[b].T) for b in range(B)]
    for c in range(NCORES):
        b, r = divmod(c, 4)
        m = dict(shared)
        m["xT"] = xT_by_b[b]
        m["xTi"] = f32c(x[b, IB * r : IB * (r + 1), :].T)
        in_maps.append(m)
    return in_maps


def _gather(results):
    full = np.empty((B, L, N, L), np.float32)
    for c in range(NCORES):
        b, r = divmod(c, 4)
        full[b, IB * r : IB * (r + 1)] = results[c]["out"]
    return full


def kernel(x, sW, sb, tW, tb, f2W, f2b, f3W, f3b, blW):
    from concourse.bass_utils import run_bass_kernel_spmd

    in_maps = _make_in_maps(dict(
        x=x, sW=sW, sb=sb, tW=tW, tb=tb, f2W=f2W, f2b=f2b,
        f3W=f3W, f3b=f3b, blW=blW,
    ))
    nc = _get_nc()
    res = run_bass_kernel_spmd(nc, in_maps, core_ids=list(range(NCORES)))
    return _gather(res.results)


# revision 19
# speedup vs baseline: 4.3528x; 1.0516x over previous
"""BiAffine layer kernel for 8 Trainium2 NeuronCores.

Reference computation (per batch b):
  s = relu(x @ sW.T + sb)                  [L, E]
  t = relu(x @ tW.T + tb)                  [L, E]
  key = (s @ blW.T).reshape(L, E, N)
  out1[i, n, l] = sum_e key[i, e, n] * t[l, e]
  su = s @ Wu.T ; tv = t @ Wv.T            (Wu, Wv = f2W[:, :E], f2W[:, E:])
  h[i, j, :] = relu(su[i] + tv[j] + f2b)
  out2[i, n, j] = sum_e h[i, j, e] * f3W[n, e] + f3b[n]
  out = out1 + out2                        [L, N, L]

Sharding: 8 cores = 2 batches x 4 blocks of 128 source positions (i).

Octet layout: one PSUM bank [128, 512] holds EIGHT i's: 4 col-groups at
32-aligned offsets, 2 i's packed per group (rows 32k + 12s + n, 8 pad
rows per group).  out1: M=32 matmuls from a zero-padded fp16 key tensor
(also initializes the bank); out2: M=24 matmuls with zero-block-padded
f3W stationaries, 4-way PE column-group concurrency throughout.

h production per octet position p = i%8:
  p=0..5 -> DVE: h' = max(tv, -su') one fp16 tensor_tensor per i with an
    interleaved-pair broadcast AP (hits the DVE 2x mode).  The dropped
    +su' makes the matmul short by sum_e f3W[n,e]su'[e,i]; that rank-1
    correction C is precomputed on the PE and folded into the final
    copy's per-partition bias.
  p=6,7 -> ACT: true h = relu(tv + su') via activation bias.
Final: one ACT copy [128,512] per octet (bias = C8[:, o] + f3b pattern),
then 4 output DMAs of [24, 512].

16-bit (fp16) surfaces: key path (blW, s), tv/su matmul operands, h,
out1/out2 matmul operands.  x and the s/t matmuls stay fp32.
"""

import sys

sys.path.insert(0, "/opt/trn_rl_repo")

import numpy as np

B, L, H, E, N = 2, 512, 768, 256, 12
EC = E // 128  # 2 e-chunks
HC = H // 128  # 6 h-chunks
IB = L // 4  # 128 i's per core
NCORES = 8
OCTS = IB // 8  # 16

_cache = {}


def build_nc():
    import concourse.bass as bass
    import concourse.tile as tile
    from concourse import bacc, mybir
    from contextlib import ExitStack

    fp32 = mybir.dt.float32
    fp16 = mybir.dt.float16
    AF = mybir.ActivationFunctionType
    ALU = mybir.AluOpType

    nc = bacc.Bacc("TRN2")

    # ---- I/O ----
    xT = nc.dram_tensor("xT", [H, L], fp32, kind="ExternalInput")
    xTi = nc.dram_tensor("xTi", [H, IB], fp32, kind="ExternalInput")
    sWT = nc.dram_tensor("sWT", [H, E], fp32, kind="ExternalInput")
    tWT = nc.dram_tensor("tWT", [H, E], fp32, kind="ExternalInput")
    WuT = nc.dram_tensor("WuT", [E, E], fp16, kind="ExternalInput")
    WvT = nc.dram_tensor("WvT", [E, E], fp16, kind="ExternalInput")
    blWT = nc.dram_tensor("blWT", [E, E * N], fp16, kind="ExternalInput")
    f3WT = nc.dram_tensor("f3WT", [E, N], fp32, kind="ExternalInput")
    f3pad = nc.dram_tensor("f3pad", [E, 48], fp16, kind="ExternalInput")
    sb_r = nc.dram_tensor("sb_r", [128, EC], fp32, kind="ExternalInput")
    tb_r = nc.dram_tensor("tb_r", [128, EC], fp32, kind="ExternalInput")
    f2b_r = nc.dram_tensor("f2b_r", [128, EC], fp32, kind="ExternalInput")
    f3b128 = nc.dram_tensor("f3b128", [128, 1], fp32, kind="ExternalInput")
    kxn01 = nc.dram_tensor("kxn01", [128, OCTS], fp32, kind="ExternalInput")
    mask8 = nc.dram_tensor("mask8", [128, 8], fp32, kind="ExternalInput")
    out = nc.dram_tensor("out", [IB, N, L], fp32, kind="ExternalOutput")

    with tile.TileContext(nc) as tc, ExitStack() as ctx:
        consts = ctx.enter_context(tc.tile_pool(name="consts", bufs=1))
        acts = ctx.enter_context(tc.tile_pool(name="acts", bufs=1))

        # ---- DMA in (spread across the 3 DMA-capable engines) ----
        def load(pool, src, shape, name, dt=fp32, eng=None):
            t = pool.tile(shape, dt, name=name)
            (eng or nc.sync).dma_start(out=t[:], in_=src)
            return t

        xT_sb = [load(consts, xT[128 * c : 128 * (c + 1), :], [128, L], f"xT{c}")
                 for c in range(HC)]
        tWT_sb = [load(consts, tWT[128 * c : 128 * (c + 1), :], [128, E], f"tWT{c}")
                  for c in range(HC)]
        xTi_sb = [load(consts, xTi[128 * c : 128 * (c + 1), :], [128, IB], f"xTi{c}",
                       eng=nc.gpsimd) for c in range(HC)]
        sWT_sb = [load(consts, sWT[128 * c : 128 * (c + 1), :], [128, E], f"sWT{c}",
                       eng=nc.gpsimd) for c in range(HC)]
        WuT_sb = [load(consts, WuT[128 * c : 128 * (c + 1), :], [128, E], f"WuT{c}",
                       dt=fp16, eng=nc.scalar) for c in range(EC)]
        WvT_sb = [load(consts, WvT[128 * c : 128 * (c + 1), :], [128, E], f"WvT{c}",
                       dt=fp16, eng=nc.scalar) for c in range(EC)]
        blWT_sb = [load(consts, blWT[128 * c : 128 * (c + 1), :], [128, E * N],
                        f"blWT{c}", dt=fp16, eng=nc.scalar) for c in range(EC)]
        f3WT_sb = [load(consts, f3WT[128 * c : 128 * (c + 1), :], [128, N], f"f3WT{c}",
                        eng=nc.gpsimd) for c in range(EC)]
        f3pad_sb = [load(consts, f3pad[128 * c : 128 * (c + 1), :], [128, 48],
                         f"f3pad{c}", dt=fp16, eng=nc.gpsimd) for c in range(EC)]
        sb_sb = load(consts, sb_r[:], [128, EC], "sb_sb")
        tb_sb = load(consts, tb_r[:], [128, EC], "tb_sb")
        f2b_sb = load(consts, f2b_r[:], [128, EC], "f2b_sb")
        f3b_sb = load(consts, f3b128[:], [128, 1], "f3b_sb")
        kxn01_sb = load(consts, kxn01[:], [128, OCTS], "kxn01_sb")
        mask8_sb = load(consts, mask8[:], [128, 8], "mask8_sb")

        # ---- persistent activations ----
        tT_sb, sT_sb, sTb_sb, suT_sb, keyE_sb = [], [], [], [], []
        for ec in range(EC):
            tT_sb.append(acts.tile([128, L], fp16, name=f"tT{ec}"))
            sT_sb.append(acts.tile([128, IB], fp32, name=f"sT{ec}"))
            sTb_sb.append(acts.tile([128, IB], fp16, name=f"sTb{ec}"))
            suT_sb.append(acts.tile([128, IB], fp32, name=f"suT{ec}"))
            # key, packed: col 32*d + 12*s + n  (i = 2d+s), pads zero
            keyE_sb.append(acts.tile([128, 32 * 64], fp16, name=f"keyE_{ec}"))
            nc.gpsimd.memset(keyE_sb[ec][:], 0.0)
        tvT2c = acts.tile([128, 2 * L], fp16, name="tvT2c")  # cols 512*ec+j
        tvT2i = acts.tile([128, 2 * L], fp16, name="tvT2i")  # cols 2*j+ec
        negsu2 = acts.tile([128, 2 * IB], fp16, name="negsu2")  # col 2i+ec
        C8sb = acts.tile([128, OCTS], fp32, name="C8sb")
        CT_sb = acts.tile([128, N], fp32, name="CT_sb")
        kxmC = acts.tile([128, 128], fp32, name="kxmC")
        nc.gpsimd.memset(kxmC[:], 0.0)

        # ---- prep ----
        with tc.tile_pool(name="prep_psum", bufs=3, space="PSUM") as pp:
            for ec in range(EC):
                # tT = relu(x @ tW.T + tb)  (fp32 matmul, fp16 out)
                ps_t = pp.tile([128, L], fp32, name="ps_t", tag="ps")
                for hc in range(HC):
                    nc.tensor.matmul(
                        ps_t[:],
                        lhsT=tWT_sb[hc][:, 128 * ec : 128 * (ec + 1)],
                        rhs=xT_sb[hc][:],
                        start=(hc == 0),
                        stop=(hc == HC - 1),
                    )
                nc.scalar.activation(tT_sb[ec][:], ps_t[:], AF.Relu,
                                     bias=tb_sb[:, ec : ec + 1])

                ps_s = pp.tile([128, L], fp32, name="ps_s", tag="ps")
                for hc in range(HC):
                    nc.tensor.matmul(
                        ps_s[:, :IB],
                        lhsT=sWT_sb[hc][:, 128 * ec : 128 * (ec + 1)],
                        rhs=xTi_sb[hc][:],
                        start=(hc == 0),
                        stop=(hc == HC - 1),
                    )
                nc.scalar.activation(sT_sb[ec][:], ps_s[:, :IB], AF.Relu,
                                     bias=sb_sb[:, ec : ec + 1])
                nc.vector.tensor_copy(out=sTb_sb[ec][:], in_=sT_sb[ec][:])

            for ec in range(EC):
                # tvT chunk (fp16 matmul) -> both layouts
                ps_tv = pp.tile([128, L], fp32, name="ps_tv", tag="ps")
                for epc in range(EC):
                    nc.tensor.matmul(
                        ps_tv[:],
                        lhsT=WvT_sb[epc][:, 128 * ec : 128 * (ec + 1)],
                        rhs=tT_sb[epc][:],
                        start=(epc == 0),
                        stop=(epc == EC - 1),
                    )
                nc.scalar.copy(tvT2c[:, L * ec : L * (ec + 1)], ps_tv[:])
                tv_i = tvT2i.rearrange("p (f c) -> p f c", c=2)
                nc.vector.tensor_copy(out=tv_i[:, :, ec], in_=ps_tv[:])

                # suT = s @ Wu.T + f2b (fp16 matmul, fp32 out)
                ps_su = pp.tile([128, L], fp32, name="ps_su", tag="ps")
                for epc in range(EC):
                    nc.tensor.matmul(
                        ps_su[:, :IB],
                        lhsT=WuT_sb[epc][:, 128 * ec : 128 * (ec + 1)],
                        rhs=sTb_sb[epc][:],
                        start=(epc == 0),
                        stop=(epc == EC - 1),
                    )
                nc.scalar.activation(suT_sb[ec][:], ps_su[:, :IB], AF.Identity,
                                     bias=f2b_sb[:, ec : ec + 1])
                ns2 = negsu2.rearrange("p (i two) -> p i two", two=2)
                nc.vector.tensor_scalar_mul(ns2[:, :, ec], suT_sb[ec][:], -1.0)

            # correction CT[i, n] = sum_e f3WT[e,n] * suT[e,i]  (fp32)
            ps_ct = pp.tile([128, L], fp32, name="ps_ct", tag="ps")
            for ec in range(EC):
                nc.tensor.matmul(
                    ps_ct[:, :N],
                    lhsT=suT_sb[ec][:],
                    rhs=f3WT_sb[ec][:],
                    start=(ec == 0),
                    stop=(ec == EC - 1),
                )
            nc.vector.tensor_copy(out=CT_sb[:], in_=ps_ct[:, :N])
            # kxmC[:, 32k+12s+n] = CT[:, n] * (i%8 == 2k+s), p<6 only
            for k in range(4):
                for s in range(2):
                    p = 2 * k + s
                    if p >= 6:
                        continue
                    nc.vector.tensor_tensor(
                        out=kxmC[:, 32 * k + 12 * s : 32 * k + 12 * s + N],
                        in0=CT_sb[:],
                        in1=mask8_sb[:, p : p + 1].broadcast_to([128, N]),
                        op=ALU.mult,
                    )
            ps_c8 = pp.tile([128, L], fp32, name="ps_c8", tag="ps")
            nc.tensor.matmul(ps_c8[:, :OCTS], lhsT=kxmC[:], rhs=kxn01_sb[:],
                             start=True, stop=True)
            nc.vector.tensor_tensor(
                out=C8sb[:], in0=ps_c8[:, :OCTS],
                in1=f3b_sb[:].broadcast_to([128, OCTS]), op=ALU.add)

            # key (fp16 matmul): keyE[ec][e, 32d+12s+n] = key[2d+s, 128ec+e, n]
            blWT3 = [blWT_sb[c].rearrange("p (e n) -> p e n", n=N) for c in range(EC)]
            keyv = [keyE_sb[c].rearrange("p (d q) -> p d q", q=32) for c in range(EC)]
            for ec in range(EC):
                for n in range(N):
                    ps_k = pp.tile([128, L], fp32, name="ps_k", tag="ps")
                    for epc in range(EC):
                        nc.tensor.matmul(
                            ps_k[:, :IB],
                            lhsT=blWT3[epc][:, 128 * ec : 128 * (ec + 1), n],
                            rhs=sTb_sb[epc][:],
                            start=(epc == 0),
                            stop=(epc == EC - 1),
                        )
                    # [128, 64, 2] strided dest (cols 32d + 12s + n)
                    nc.vector.tensor_copy(
                        out=keyv[ec][:, :, n : n + 13 : 12],
                        in_=ps_k[:, :IB].rearrange("p (d s) -> p d s", s=2),
                    )

        # ---- main loop over octets ----
        hp = ctx.enter_context(tc.tile_pool(name="hp", bufs=20))
        outp = ctx.enter_context(tc.tile_pool(name="outp", bufs=4))
        mp = ctx.enter_context(tc.tile_pool(name="main_psum", bufs=5, space="PSUM"))

        for o in range(OCTS):
            ps = mp.tile([128, L], fp32, name="ps")
            # out1: M=32 per (duo, ec); ec0 initializes the full bank
            for ec in range(EC):
                for k in range(4):
                    d = 4 * o + k
                    nc.tensor.matmul(
                        ps[32 * k : 32 * k + 32, :],
                        lhsT=keyE_sb[ec][:, 32 * d : 32 * d + 32],
                        rhs=tT_sb[ec][:],
                        start=(ec == 0),
                        stop=False,
                        tile_position=(0, 32 * k),
                        skip_group_check=True,
                    )
            # h production: p = 0..5 DVE (h'), p = 6,7 ACT (true h)
            hs = {}
            for p in range(8):
                i = 8 * o + p
                if p < 6:
                    h2 = hp.tile([128, 2 * L], fp16, name="h2", tag="h")
                    in1 = negsu2[:, 2 * i : 2 * i + 2].unsqueeze(1)\
                        .broadcast_to([128, L, 2])
                    nc.vector.tensor_tensor(
                        out=h2[:].rearrange("p (f c) -> p f c", c=2),
                        in0=tvT2i[:].rearrange("p (f c) -> p f c", c=2),
                        in1=in1, op=ALU.max)
                    h2v = h2.rearrange("p (f c) -> p f c", c=2)
                    for ec in range(EC):
                        hs[(p, ec)] = h2v[:, :, ec]
                else:
                    for ec in range(EC):
                        ha = hp.tile([128, L], fp16, name="ha", tag="h")
                        nc.scalar.activation(ha[:], tvT2c[:, L * ec : L * (ec + 1)],
                                             AF.Relu, bias=suT_sb[ec][:, i : i + 1])
                        hs[(p, ec)] = ha[:]
            # out2: M=24 zero-block-padded f3 stationaries
            for ec in range(EC):
                for p in range(8):
                    k, s = divmod(p, 2)
                    nc.tensor.matmul(
                        ps[32 * k : 32 * k + 24, :],
                        lhsT=f3pad_sb[ec][:, 24 * s : 24 * s + 24],
                        rhs=hs[(p, ec)],
                        start=False,
                        stop=(ec == EC - 1),
                        tile_position=(0, 32 * k),
                        skip_group_check=True,
                    )
            # finalize: one copy per octet with per-partition bias
            ob = outp.tile([128, L], fp32, name="ob")
            nc.scalar.activation(ob[:], ps[:], AF.Identity,
                                 bias=C8sb[:, o : o + 1])
            for k in range(4):
                dma_eng = nc.sync if k % 2 == 0 else nc.gpsimd
                dma_eng.dma_start(out=out[8 * o + 2 * k : 8 * o + 2 * k + 2, :, :],
                                  in_=ob[32 * k : 32 * k + 24, :])

    nc.compile()
    return nc


def _get_nc():
    if "nc" not in _cache:
        _cache["nc"] = build_nc()
    return _cache["nc"]


def _make_in_maps(inputs):
    x = np.asarray(inputs["x"], np.float32)
    f32c = lambda a: np.ascontiguousarray(np.asarray(a, np.float32))
    f16c = lambda a: np.ascontiguousarray(np.asarray(a, np.float32).astype(np.float16))

    f2W = np.asarray(inputs["f2W"], np.float32)
    f3WT = f32c(inputs["f3W"].T)
    f3pad = np.zeros((E, 48), np.float32)
    for s in range(2):
        # slice s covers psum rows 32k..32k+24; i with s=i%2 lands at +12*s
        f3pad[:, 24 * s + 12 * s : 24 * s + 12 * s + N] = f3WT
    kxn01 = np.zeros((128, OCTS), np.float32)
    mask8 = np.zeros((128, 8), np.float32)
    for i in range(128):
        if i % 8 < 6:
            kxn01[i, i // 8] = 1.0
        mask8[i, i % 8] = 1.0
    f3b128 = np.zeros((128, 1), np.float32)
    for k in range(4):
        for s in range(2):
            f3b128[32 * k + 12 * s : 32 * k + 12 * s + N, 0] = \
                np.asarray(inputs["f3b"], np.float32)
    shared = {
        "sWT": f32c(inputs["sW"].T),
        "tWT": f32c(inputs["tW"].T),
        "WuT": f16c(f2W[:, :E].T),
        "WvT": f16c(f2W[:, E:].T),
        "blWT": f16c(inputs["blW"].T),
        "f3WT": f3WT,
        "f3pad": np.ascontiguousarray(f3pad.astype(np.float16)),
        "sb_r": f32c(np.asarray(inputs["sb"], np.float32).reshape(EC, 128).T),
        "tb_r": f32c(np.asarray(inputs["tb"], np.float32).reshape(EC, 128).T),
        "f2b_r": f32c(np.asarray(inputs["f2b"], np.float32).reshape(EC, 128).T),
        "f3b128": f3b128,
        "kxn01": kxn01,
        "mask8": mask8,
    }

    in_maps = []
    xT_by_b = [f32c(x[b].T) for b in range(B)]
    for c in range(NCORES):
        b, r = divmod(c, 4)
        m = dict(shared)
        m["xT"] = xT_by_b[b]
        m["xTi"] = f32c(x[b, IB * r : IB * (r + 1), :].T)
        in_maps.append(m)
    return in_maps


def _gather(results):
    full = np.empty((B, L, N, L), np.float32)
    for c in range(NCORES):
        b, r = divmod(c, 4)
        full[b, IB * r : IB * (r + 1)] = results[c]["out"]
    return full


def kernel(x, sW, sb, tW, tb, f2W, f2b, f3W, f3b, blW):
    from concourse.bass_utils import run_bass_kernel_spmd

    in_maps = _make_in_maps(dict(
        x=x, sW=sW, sb=sb, tW=tW, tb=tb, f2W=f2W, f2b=f2b,
        f3W=f3W, f3b=f3b, blW=blW,
    ))
    nc = _get_nc()
    res = run_bass_kernel_spmd(nc, in_maps, core_ids=list(range(NCORES)))
    return _gather(res.results)


# revision 20
# speedup vs baseline: 4.6312x; 1.0640x over previous
"""BiAffine layer kernel for 8 Trainium2 NeuronCores.

Reference computation (per batch b):
  s = relu(x @ sW.T + sb)                  [L, E]
  t = relu(x @ tW.T + tb)                  [L, E]
  key = (s @ blW.T).reshape(L, E, N)
  out1[i, n, l] = sum_e key[i, e, n] * t[l, e]
  su = s @ Wu.T ; tv = t @ Wv.T            (Wu, Wv = f2W[:, :E], f2W[:, E:])
  h[i, j, :] = relu(su[i] + tv[j] + f2b)
  out2[i, n, j] = sum_e h[i, j, e] * f3W[n, e] + f3b[n]
  out = out1 + out2                        [L, N, L]

Sharding: 8 cores = 2 batches x 4 blocks of 128 source positions (i).

Octet layout: one PSUM bank [128, 512] holds EIGHT i's: 4 col-groups at
32-aligned offsets, 2 i's packed per group (rows 32k + 12s + n, 8 pad
rows per group).  out1: M=32 matmuls from a zero-padded fp16 key tensor
(also initializes the bank); out2: M=24 matmuls with zero-block-padded
f3W stationaries, 4-way PE column-group concurrency throughout.

h production per octet position p = i%8:
  p=0..5 -> DVE: h' = max(tv, -su') one fp16 tensor_tensor per i with an
    interleaved-pair broadcast AP (hits the DVE 2x mode).  The dropped
    +su' makes the matmul short by sum_e f3W[n,e]su'[e,i]; that rank-1
    correction C is precomputed on the PE and folded into the final
    copy's per-partition bias.
  p=6,7 -> ACT: true h = relu(tv + su') via activation bias.
Final: one ACT copy [128,512] per octet (software-pipelined one octet
behind the matmuls), then 4 output DMAs of [24, 512].

DMA-instruction issue costs ~600ns regardless of size, so every
multi-chunk tensor is loaded with ONE DMA from a host-prepacked layout
(chunk-major in the free dim), and all small tensors ride in one
"misc" tensor per dtype.
"""

import sys

sys.path.insert(0, "/opt/trn_rl_repo")

import numpy as np

B, L, H, E, N = 2, 512, 768, 256, 12
EC = E // 128  # 2 e-chunks
HC = H // 128  # 6 h-chunks
IB = L // 4  # 128 i's per core
NCORES = 8
OCTS = IB // 8  # 16

# misc fp32 tensor column layout: [sb(2) tb(2) f2b(2) f3b128(1) kxn01(16)
#                                  mask8(8) f3WT(24)]
MISC_W = 2 + 2 + 2 + 1 + OCTS + 8 + 2 * N

_cache = {}


def build_nc():
    import concourse.bass as bass
    import concourse.tile as tile
    from concourse import bacc, mybir
    from contextlib import ExitStack

    fp32 = mybir.dt.float32
    fp16 = mybir.dt.float16
    AF = mybir.ActivationFunctionType
    ALU = mybir.AluOpType

    nc = bacc.Bacc("TRN2")

    # ---- I/O (all multi-chunk tensors prepacked chunk-major on host) ----
    xTm = nc.dram_tensor("xTm", [128, HC * L], fp32, kind="ExternalInput")
    tWTm = nc.dram_tensor("tWTm", [128, HC * E], fp32, kind="ExternalInput")
    xTim = nc.dram_tensor("xTim", [128, HC * IB], fp32, kind="ExternalInput")
    sWTm = nc.dram_tensor("sWTm", [128, HC * E], fp32, kind="ExternalInput")
    WuTm = nc.dram_tensor("WuTm", [128, EC * E], fp16, kind="ExternalInput")
    WvTm = nc.dram_tensor("WvTm", [128, EC * E], fp16, kind="ExternalInput")
    blWTm = nc.dram_tensor("blWTm", [128, EC * E * N], fp16, kind="ExternalInput")
    f3padm = nc.dram_tensor("f3padm", [128, EC * 48], fp16, kind="ExternalInput")
    misc = nc.dram_tensor("misc", [128, MISC_W], fp32, kind="ExternalInput")
    out = nc.dram_tensor("out", [IB, N, L], fp32, kind="ExternalOutput")

    with tile.TileContext(nc) as tc, ExitStack() as ctx:
        consts = ctx.enter_context(tc.tile_pool(name="consts", bufs=1))
        acts = ctx.enter_context(tc.tile_pool(name="acts", bufs=1))

        def load(src, shape, name, dt=fp32, eng=None):
            t = consts.tile(shape, dt, name=name)
            (eng or nc.sync).dma_start(out=t[:], in_=src)
            return t

        # queue order matters: first-needed first per queue
        xT_m = load(xTm[:], [128, HC * L], "xT_m")
        tWT_m = load(tWTm[:], [128, HC * E], "tWT_m")
        xTi_m = load(xTim[:], [128, HC * IB], "xTi_m", eng=nc.gpsimd)
        sWT_m = load(sWTm[:], [128, HC * E], "sWT_m", eng=nc.gpsimd)
        misc_sb = load(misc[:], [128, MISC_W], "misc_sb", eng=nc.gpsimd)
        WuT_m = load(WuTm[:], [128, EC * E], "WuT_m", dt=fp16, eng=nc.scalar)
        WvT_m = load(WvTm[:], [128, EC * E], "WvT_m", dt=fp16, eng=nc.scalar)
        f3pad_m = load(f3padm[:], [128, EC * 48], "f3pad_m", dt=fp16, eng=nc.scalar)
        blWT_m = load(blWTm[:], [128, EC * E * N], "blWT_m", dt=fp16, eng=nc.scalar)

        xT_sb = [xT_m[:, L * c : L * (c + 1)] for c in range(HC)]
        tWT_sb = [tWT_m[:, E * c : E * (c + 1)] for c in range(HC)]
        xTi_sb = [xTi_m[:, IB * c : IB * (c + 1)] for c in range(HC)]
        sWT_sb = [sWT_m[:, E * c : E * (c + 1)] for c in range(HC)]
        WuT_sb = [WuT_m[:, E * c : E * (c + 1)] for c in range(EC)]
        WvT_sb = [WvT_m[:, E * c : E * (c + 1)] for c in range(EC)]
        blWT_sb = [blWT_m[:, E * N * c : E * N * (c + 1)] for c in range(EC)]
        f3pad_sb = [f3pad_m[:, 48 * c : 48 * (c + 1)] for c in range(EC)]
        o_ = 0
        sb_sb = misc_sb[:, o_ : o_ + 2]; o_ += 2
        tb_sb = misc_sb[:, o_ : o_ + 2]; o_ += 2
        f2b_sb = misc_sb[:, o_ : o_ + 2]; o_ += 2
        f3b_sb = misc_sb[:, o_ : o_ + 1]; o_ += 1
        kxn01_sb = misc_sb[:, o_ : o_ + OCTS]; o_ += OCTS
        mask8_sb = misc_sb[:, o_ : o_ + 8]; o_ += 8
        f3WT_sb = [misc_sb[:, o_ + N * c : o_ + N * (c + 1)] for c in range(EC)]

        # ---- persistent activations ----
        tT_sb, sT_sb, sTb_sb, suT_sb, keyE_sb = [], [], [], [], []
        for ec in range(EC):
            tT_sb.append(acts.tile([128, L], fp16, name=f"tT{ec}"))
            sT_sb.append(acts.tile([128, IB], fp32, name=f"sT{ec}"))
            sTb_sb.append(acts.tile([128, IB], fp16, name=f"sTb{ec}"))
            suT_sb.append(acts.tile([128, IB], fp32, name=f"suT{ec}"))
            # key, packed: col 32*d + 12*s + n  (i = 2d+s), pads zero
            keyE_sb.append(acts.tile([128, 32 * 64], fp16, name=f"keyE_{ec}"))
            nc.gpsimd.memset(keyE_sb[ec][:], 0.0)
        tvT2c = acts.tile([128, 2 * L], fp16, name="tvT2c")  # cols 512*ec+j
        tvT2i = acts.tile([128, 2 * L], fp16, name="tvT2i")  # cols 2*j+ec
        negsu2 = acts.tile([128, 2 * IB], fp16, name="negsu2")  # col 2i+ec
        C8sb = acts.tile([128, OCTS], fp32, name="C8sb")
        CT_sb = acts.tile([128, N], fp32, name="CT_sb")
        kxmC = acts.tile([128, 128], fp32, name="kxmC")
        nc.gpsimd.memset(kxmC[:], 0.0)

        # ---- prep ----
        with tc.tile_pool(name="prep_psum", bufs=3, space="PSUM") as pp:
            for ec in range(EC):
                # tT = relu(x @ tW.T + tb)  (fp32 matmul, fp16 out)
                ps_t = pp.tile([128, L], fp32, name="ps_t", tag="ps")
                for hc in range(HC):
                    nc.tensor.matmul(
                        ps_t[:],
                        lhsT=tWT_sb[hc][:, 128 * ec : 128 * (ec + 1)],
                        rhs=xT_sb[hc],
                        start=(hc == 0),
                        stop=(hc == HC - 1),
                    )
                nc.scalar.activation(tT_sb[ec][:], ps_t[:], AF.Relu,
                                     bias=tb_sb[:, ec : ec + 1])

                ps_s = pp.tile([128, L], fp32, name="ps_s", tag="ps")
                for hc in range(HC):
                    nc.tensor.matmul(
                        ps_s[:, :IB],
                        lhsT=sWT_sb[hc][:, 128 * ec : 128 * (ec + 1)],
                        rhs=xTi_sb[hc],
                        start=(hc == 0),
                        stop=(hc == HC - 1),
                    )
                nc.scalar.activation(sT_sb[ec][:], ps_s[:, :IB], AF.Relu,
                                     bias=sb_sb[:, ec : ec + 1])
                nc.vector.tensor_copy(out=sTb_sb[ec][:], in_=sT_sb[ec][:])

            for ec in range(EC):
                # tvT chunk (fp16 matmul) -> both layouts
                ps_tv = pp.tile([128, L], fp32, name="ps_tv", tag="ps")
                for epc in range(EC):
                    nc.tensor.matmul(
                        ps_tv[:],
                        lhsT=WvT_sb[epc][:, 128 * ec : 128 * (ec + 1)],
                        rhs=tT_sb[epc][:],
                        start=(epc == 0),
                        stop=(epc == EC - 1),
                    )
                nc.scalar.copy(tvT2c[:, L * ec : L * (ec + 1)], ps_tv[:])
                tv_i = tvT2i.rearrange("p (f c) -> p f c", c=2)
                nc.vector.tensor_copy(out=tv_i[:, :, ec], in_=ps_tv[:])

                # suT = s @ Wu.T + f2b (fp16 matmul, fp32 out)
                ps_su = pp.tile([128, L], fp32, name="ps_su", tag="ps")
                for epc in range(EC):
                    nc.tensor.matmul(
                        ps_su[:, :IB],
                        lhsT=WuT_sb[epc][:, 128 * ec : 128 * (ec + 1)],
                        rhs=sTb_sb[epc][:],
                        start=(epc == 0),
                        stop=(epc == EC - 1),
                    )
                nc.scalar.activation(suT_sb[ec][:], ps_su[:, :IB], AF.Identity,
                                     bias=f2b_sb[:, ec : ec + 1])
                ns2 = negsu2.rearrange("p (i two) -> p i two", two=2)
                nc.vector.tensor_scalar_mul(ns2[:, :, ec], suT_sb[ec][:], -1.0)

            # correction CT[i, n] = sum_e f3WT[e,n] * suT[e,i]  (fp32)
            ps_ct = pp.tile([128, L], fp32, name="ps_ct", tag="ps")
            for ec in range(EC):
                nc.tensor.matmul(
                    ps_ct[:, :N],
                    lhsT=suT_sb[ec][:],
                    rhs=f3WT_sb[ec],
                    start=(ec == 0),
                    stop=(ec == EC - 1),
                )
            nc.vector.tensor_copy(out=CT_sb[:], in_=ps_ct[:, :N])
            # kxmC[:, 32k+12s+n] = CT[:, n] * (i%8 == 2k+s), p<6 only
            for k in range(4):
                for s in range(2):
                    p = 2 * k + s
                    if p >= 6:
                        continue
                    nc.vector.tensor_tensor(
                        out=kxmC[:, 32 * k + 12 * s : 32 * k + 12 * s + N],
                        in0=CT_sb[:],
                        in1=mask8_sb[:, p : p + 1].broadcast_to([128, N]),
                        op=ALU.mult,
                    )
            ps_c8 = pp.tile([128, L], fp32, name="ps_c8", tag="ps")
            nc.tensor.matmul(ps_c8[:, :OCTS], lhsT=kxmC[:], rhs=kxn01_sb,
                             start=True, stop=True)
            nc.vector.tensor_tensor(
                out=C8sb[:], in0=ps_c8[:, :OCTS],
                in1=f3b_sb.broadcast_to([128, OCTS]), op=ALU.add)

            # key (fp16 matmul): keyE[ec][e, 32d+12s+n] = key[2d+s, 128ec+e, n]
            blWT3 = [blWT_sb[c].rearrange("p (e n) -> p e n", n=N) for c in range(EC)]
            keyv = [keyE_sb[c].rearrange("p (d q) -> p d q", q=32) for c in range(EC)]
            for ec in range(EC):
                for n in range(N):
                    ps_k = pp.tile([128, L], fp32, name="ps_k", tag="ps")
                    for epc in range(EC):
                        nc.tensor.matmul(
                            ps_k[:, :IB],
                            lhsT=blWT3[epc][:, 128 * ec : 128 * (ec + 1), n],
                            rhs=sTb_sb[epc][:],
                            start=(epc == 0),
                            stop=(epc == EC - 1),
                        )
                    # [128, 64, 2] strided dest (cols 32d + 12s + n)
                    src = ps_k[:, :IB].rearrange("p (d s) -> p d s", s=2)
                    dst = keyv[ec][:, :, n : n + 13 : 12]
                    if n % 2 == 0:
                        nc.vector.tensor_copy(out=dst, in_=src)
                    else:
                        nc.scalar.copy(dst, src)

        # ---- main loop over octets (final copy pipelined 1 octet back) ----
        hp = ctx.enter_context(tc.tile_pool(name="hp", bufs=20))
        outp = ctx.enter_context(tc.tile_pool(name="outp", bufs=4))
        mp = ctx.enter_context(tc.tile_pool(name="main_psum", bufs=5, space="PSUM"))

        pending = None  # (psum_tile, octet)

        def flush(pending):
            ps_prev, o_prev = pending
            ob = outp.tile([128, L], fp32, name="ob")
            nc.scalar.activation(ob[:], ps_prev[:], AF.Identity,
                                 bias=C8sb[:, o_prev : o_prev + 1])
            for k in range(4):
                dma_eng = nc.sync if k % 2 == 0 else nc.gpsimd
                dma_eng.dma_start(
                    out=out[8 * o_prev + 2 * k : 8 * o_prev + 2 * k + 2, :, :],
                    in_=ob[32 * k : 32 * k + 24, :])

        for o in range(OCTS):
            ps = mp.tile([128, L], fp32, name="ps")
            # out1: M=32 per (duo, ec); ec0 initializes the full bank
            for ec in range(EC):
                for k in range(4):
                    d = 4 * o + k
                    nc.tensor.matmul(
                        ps[32 * k : 32 * k + 32, :],
                        lhsT=keyE_sb[ec][:, 32 * d : 32 * d + 32],
                        rhs=tT_sb[ec][:],
                        start=(ec == 0),
                        stop=False,
                        tile_position=(0, 32 * k),
                        skip_group_check=True,
                    )
            # h production: p = 0..5 DVE (h'), p = 6,7 ACT (true h)
            hs = {}
            for p in range(8):
                i = 8 * o + p
                if p < 6:
                    h2 = hp.tile([128, 2 * L], fp16, name="h2", tag="h")
                    in1 = negsu2[:, 2 * i : 2 * i + 2].unsqueeze(1)\
                        .broadcast_to([128, L, 2])
                    nc.vector.tensor_tensor(
                        out=h2[:].rearrange("p (f c) -> p f c", c=2),
                        in0=tvT2i[:].rearrange("p (f c) -> p f c", c=2),
                        in1=in1, op=ALU.max)
                    h2v = h2.rearrange("p (f c) -> p f c", c=2)
                    for ec in range(EC):
                        hs[(p, ec)] = h2v[:, :, ec]
                else:
                    for ec in range(EC):
                        ha = hp.tile([128, L], fp16, name="ha", tag="h")
                        nc.scalar.activation(ha[:], tvT2c[:, L * ec : L * (ec + 1)],
                                             AF.Relu, bias=suT_sb[ec][:, i : i + 1])
                        hs[(p, ec)] = ha[:]
            # out2: M=24 zero-block-padded f3 stationaries; emission order
            # rotates col-groups for PE tile concurrency
            for ec in range(EC):
                for p in (0, 2, 4, 6, 1, 3, 5, 7):
                    k, s = divmod(p, 2)
                    nc.tensor.matmul(
                        ps[32 * k : 32 * k + 24, :],
                        lhsT=f3pad_sb[ec][:, 24 * s : 24 * s + 24],
                        rhs=hs[(p, ec)],
                        start=False,
                        stop=(ec == EC - 1),
                        tile_position=(0, 32 * k),
                        skip_group_check=True,
                    )
            if pending is not None:
                flush(pending)
            pending = (ps, o)
        flush(pending)

    nc.compile()
    return nc


def _get_nc():
    if "nc" not in _cache:
        _cache["nc"] = build_nc()
    return _cache["nc"]


def _chunk_major(a, nchunks):
    # [128*nchunks, W] -> [128, nchunks*W] with chunk-major free layout
    W = a.shape[1]
    return np.ascontiguousarray(
        a.reshape(nchunks, 128, W).transpose(1, 0, 2).reshape(128, nchunks * W))


def _make_in_maps(inputs):
    x = np.asarray(inputs["x"], np.float32)
    f32 = lambda a: np.asarray(a, np.float32)

    f2W = f32(inputs["f2W"])
    f3WT = f32(inputs["f3W"]).T  # [E, N]
    f3pad = np.zeros((E, 48), np.float32)
    for s in range(2):
        # slice s covers psum rows 32k..32k+24; i with s=i%2 lands at +12*s
        f3pad[:, 24 * s + 12 * s : 24 * s + 12 * s + N] = f3WT

    misc = np.zeros((128, MISC_W), np.float32)
    o_ = 0
    misc[:, o_ : o_ + 2] = f32(inputs["sb"]).reshape(EC, 128).T; o_ += 2
    misc[:, o_ : o_ + 2] = f32(inputs["tb"]).reshape(EC, 128).T; o_ += 2
    misc[:, o_ : o_ + 2] = f32(inputs["f2b"]).reshape(EC, 128).T; o_ += 2
    for k in range(4):
        for s in range(2):
            misc[32 * k + 12 * s : 32 * k + 12 * s + N, o_] = f32(inputs["f3b"])
    o_ += 1
    for i in range(128):
        if i % 8 < 6:
            misc[i, o_ + i // 8] = 1.0
    o_ += OCTS
    for i in range(128):
        misc[i, o_ + i % 8] = 1.0
    o_ += 8
    misc[:, o_:] = _chunk_major(f3WT, EC)

    shared = {
        "sWTm": _chunk_major(f32(inputs["sW"]).T, HC),
        "tWTm": _chunk_major(f32(inputs["tW"]).T, HC),
        "WuTm": _chunk_major(f2W[:, :E].T, EC).astype(np.float16),
        "WvTm": _chunk_major(f2W[:, E:].T, EC).astype(np.float16),
        "blWTm": _chunk_major(f32(inputs["blW"]).T, EC).astype(np.float16),
        "f3padm": _chunk_major(f3pad, EC).astype(np.float16),
        "misc": misc,
    }

    in_maps = []
    for c in range(NCORES):
        b, r = divmod(c, 4)
        m = dict(shared)
        m["xTm"] = _chunk_major(np.ascontiguousarray(x[b].T), HC)
        m["xTim"] = _chunk_major(
            np.ascontiguousarray(x[b, IB * r : IB * (r + 1), :].T), HC)
        in_maps.append(m)
    return in_maps


def _gather(results):
    full = np.empty((B, L, N, L), np.float32)
    for c in range(NCORES):
        b, r = divmod(c, 4)
        full[b, IB * r : IB * (r + 1)] = results[c]["out"]
    return full


def kernel(x, sW, sb, tW, tb, f2W, f2b, f3W, f3b, blW):
    from concourse.bass_utils import run_bass_kernel_spmd

    in_maps = _make_in_maps(dict(
        x=x, sW=sW, sb=sb, tW=tW, tb=tb, f2W=f2W, f2b=f2b,
        f3W=f3W, f3b=f3b, blW=blW,
    ))
    nc = _get_nc()
    res = run_bass_kernel_spmd(nc, in_maps, core_ids=list(range(NCORES)))
    return _gather(res.results)
